# revision 1
# baseline (speedup 1.0000x reference)
"""Trainium2 Bass kernel for a 6-layer post-LN transformer encoder.

Sharding: data-parallel over batch — B=8, one batch element per NeuronCore,
no collectives.  Each core runs the full 6-layer encoder on its [S, D] slice.

Device-side layout: activations are kept feature-major ([D, S], "xT") in SBUF
so that every matmul can use the natural input-major weights as the stationary
(lhsT) operand and PE contracts over the partition dim:

  out[m, n] = sum_k lhsT[k, m] * rhs[k, n]

Attention is computed transposed (scoresT[t, s]) so softmax needs no
transposes: denominators fall out of a ones-column in the ctx matmul, and the
per-column 1/denom broadcast is a k=1 matmul on PE.

All matmul operands are float32r (fp32 rounded to 11 mantissa bits): the PE
streams fp32r at 1 row/cycle vs 4 for plain fp32 — 4x matmul throughput.
Engine writes to fp32r tiles round on write; DMA'd fp32r data is pre-rounded
on the host.
"""

import numpy as np

L, H, D, DK, DFF = 6, 8, 512, 64, 2048
B, S = 8, 1024
EPS = 1e-5
P = 128
NDT = D // P        # 4  d-tiles
NST = S // P        # 8  s/t-tiles
NFT = DFF // P      # 16 dff-tiles
NPAIR = H // 2      # 4  head pairs
NH = S // 512       # 2  n-halves (512-wide fp32 matmul free dim)
FCH = 2             # W1 streamed in chunks of 2 dff-tiles
SCALE = 1.0 / np.sqrt(np.float32(DK))

_CACHE = {}


def _round_fp32r(a: np.ndarray) -> np.ndarray:
    """Round fp32 to the fp32r grid (11 mantissa bits), round-to-nearest-even."""
    u = np.ascontiguousarray(a, dtype=np.float32).view(np.uint32)
    r = (u + np.uint32(0x7FF) + ((u >> np.uint32(12)) & np.uint32(1))) & np.uint32(
        0xFFFFF000
    )
    return r.view(np.float32)


def _build_nc():
    import concourse.bass as bass
    import concourse.bacc as bacc
    import concourse.tile as tile
    from concourse import mybir

    fp32 = mybir.dt.float32
    fp32r = mybir.dt.float32r
    AF = mybir.ActivationFunctionType
    OP = mybir.AluOpType

    class _Bacc(bacc.Bacc):
        # Exp (softmax) and Ln (layernorm rstd) live in different default
        # activation-table sets, causing ~50 table-load thrashes (~2.7us
        # each). Restrict both to natural_log_exp_and_others (which holds
        # both) so one load serves the whole kernel. Positional set ids are
        # preserved; only the function->set resolution changes.
        def insert_act_table_loads(self):
            from concourse.hw_specs import get_activation_tables
            import bass_rust as _bass_rust

            has_act = any(
                isinstance(i, mybir.InstActivation)
                for b in self.main_func.blocks
                for i in b.instructions
            )
            if not has_act:
                return
            AF2 = mybir.ActivationFunctionType
            tables = []
            for name, fns in get_activation_tables(self.m.arch).items():
                if name != "natural_log_exp_and_others":
                    fns = fns - {AF2.Exp, AF2.Ln}
                tables.append((name, fns))
            _bass_rust.insert_act_table_loads(self, tables)

    nc = _Bacc()

    def mm(out, lhsT, rhs, **kw):
        # float32r operands: PE streams 1 row/cycle (vs 4 for plain fp32)
        return nc.tensor.matmul(
            out, lhsT.bitcast(fp32r), rhs.bitcast(fp32r), **kw
        )

    def f(ap):
        # view a float32r tile as plain fp32 for DVE/ACT reads
        return ap.bitcast(fp32)

    x_d = nc.declare_dram_parameter("x", [NDT, P, S], fp32r, isOutput=False)
    wq_d = nc.declare_dram_parameter(
        "wq", [L, P, NDT, NPAIR, P], fp32r, isOutput=False
    )
    wk_d = nc.declare_dram_parameter(
        "wk", [L, P, NDT, NPAIR, P], fp32r, isOutput=False
    )
    wv_d = nc.declare_dram_parameter("wv", [L, P, NDT, H * DK], fp32r, isOutput=False)
    wo_d = nc.declare_dram_parameter("wo", [L, 64, H, NDT, P], fp32r, isOutput=False)
    w1_d = nc.declare_dram_parameter(
        "w1", [L, NFT // FCH, P, NDT, FCH, P], fp32r, isOutput=False
    )
    w2_d = nc.declare_dram_parameter(
        "w2", [L, NFT // 4, P, 4, NDT, P], fp32r, isOutput=False
    )
    g1_d = nc.declare_dram_parameter("g1", [L, P, NDT], fp32, isOutput=False)
    be1_d = nc.declare_dram_parameter("be1", [L, P, NDT], fp32, isOutput=False)
    g2_d = nc.declare_dram_parameter("g2", [L, P, NDT], fp32, isOutput=False)
    be2_d = nc.declare_dram_parameter("be2", [L, P, NDT], fp32, isOutput=False)
    b1_d = nc.declare_dram_parameter("b1", [L, P, NFT], fp32, isOutput=False)
    b2_d = nc.declare_dram_parameter("b2", [L, P, NDT], fp32, isOutput=False)
    ones_d = nc.declare_dram_parameter("ones", [P, P], fp32r, isOutput=False)
    out_d = nc.declare_dram_parameter("out", [NDT, P, S], fp32, isOutput=True)

    with tile.TileContext(nc) as tc:
        from contextlib import ExitStack

        with ExitStack() as ctx:
            ec = ctx.enter_context
            ec(
                nc.allow_low_precision(
                    reason="fp32r matmul operands; fp32 PSUM accumulation"
                )
            )
            # --- SBUF pools ---
            const_p = ec(tc.tile_pool(name="const", bufs=1))
            wts_p = ec(tc.tile_pool(name="wts", bufs=1))
            w1c_p = ec(tc.tile_pool(name="w1c", bufs=2))
            xt_p = ec(tc.tile_pool(name="xt", bufs=4))
            qk_p = ec(tc.tile_pool(name="qk", bufs=2))
            v_p = ec(tc.tile_pool(name="v", bufs=8))
            exp_p = ec(tc.tile_pool(name="exp", bufs=5))
            ctx_p = ec(tc.tile_pool(name="ctxp", bufs=8))
            mha_p = ec(tc.tile_pool(name="mha", bufs=4))
            ff1_p = ec(tc.tile_pool(name="ff1", bufs=4))
            ysq_p = ec(tc.tile_pool(name="ysq", bufs=2))
            bcs_p = ec(tc.tile_pool(name="bcs", bufs=2))
            rows_p = ec(tc.tile_pool(name="rows", bufs=1))
            # --- PSUM pools: 2 + 4 + 2 = 8 banks ---
            # sc: scores / LN-stats / broadcast tiles (short-lived)
            # acc: long-lived accumulation groups (attention ctx A+B;
            #      FFN2 needs all 4 mt-groups concurrent so ff1 tiles release)
            # mm: short-lived k<=4 matmul outputs (QKV/V/Wo/FFN1)
            pp_sc = ec(tc.tile_pool(name="pp_sc", bufs=2, space="PSUM"))
            pp_acc = ec(tc.tile_pool(name="pp_acc", bufs=4, space="PSUM"))
            pp_mm = ec(tc.tile_pool(name="pp_mm", bufs=2, space="PSUM"))

            # ones come from DRAM so the fp32r data counts as pre-rounded
            ones_full = const_p.tile([P, P], fp32r)
            nc.sync.dma_start(out=ones_full, in_=ones_d[:, :])
            ones_col = ones_full[:, 0:1]
            zero_col = const_p.tile([P, 1], fp32)
            nc.vector.memset(zero_col, 0.0)
            eps_col = const_p.tile([P, 1], fp32)
            nc.vector.memset(eps_col, float(EPS))

            # layer-0 input
            xt = []
            for dt in range(NDT):
                t = xt_p.tile([P, S], fp32r, tag="xt")
                nc.sync.dma_start(out=t, in_=x_d[dt])
                xt.append(t)

            for l in range(L):
                # ---------------- weight loads (released per-layer; bufs=1
                # pools serialize against last use of previous layer) --------
                wq_t = wts_p.tile([P, NDT, NPAIR, P], fp32r, tag="wq")
                nc.sync.dma_start(out=wq_t, in_=wq_d[l])
                wk_t = wts_p.tile([P, NDT, NPAIR, P], fp32r, tag="wk")
                nc.sync.dma_start(out=wk_t, in_=wk_d[l])
                wv_t = wts_p.tile([P, NDT, H * DK], fp32r, tag="wv")
                nc.sync.dma_start(out=wv_t, in_=wv_d[l])
                wo_t = wts_p.tile([64, H, NDT, P], fp32r, tag="wo")
                nc.sync.dma_start(out=wo_t, in_=wo_d[l])
                g1_t = wts_p.tile([P, NDT], fp32, tag="g1")
                nc.sync.dma_start(out=g1_t, in_=g1_d[l])
                be1_t = wts_p.tile([P, NDT], fp32, tag="be1")
                nc.sync.dma_start(out=be1_t, in_=be1_d[l])
                g2_t = wts_p.tile([P, NDT], fp32, tag="g2")
                nc.sync.dma_start(out=g2_t, in_=g2_d[l])
                be2_t = wts_p.tile([P, NDT], fp32, tag="be2")
                nc.sync.dma_start(out=be2_t, in_=be2_d[l])
                b1_t = wts_p.tile([P, NFT], fp32, tag="b1")
                nc.sync.dma_start(out=b1_t, in_=b1_d[l])
                b2_t = wts_p.tile([P, NDT], fp32, tag="b2")
                nc.sync.dma_start(out=b2_t, in_=b2_d[l])

                # ---------------- V = x @ Wv  (row-major [t, (h,dk)]) -------
                v_tiles = []
                for st in range(NST):
                    vt = v_p.tile([P, H, DK + 1], fp32r, tag="v")
                    nc.sync.dma_start(out=vt[:, :, DK], in_=ones_d[:, 0:H])
                    ps = pp_mm.tile([P, 512], fp32, tag="mm")
                    for dt in range(NDT):
                        mm(
                            ps,
                            xt[dt][:, st * P : (st + 1) * P],
                            wv_t[:, dt, :],
                            start=(dt == 0),
                            stop=(dt == NDT - 1),
                        )
                    nc.vector.tensor_copy(
                        vt[:, :, 0:DK], ps.rearrange("p (h k) -> p h k", h=H)
                    )
                    v_tiles.append(vt)

                # ---------------- QKV (per head-pair, JIT) + attention ------
                qt = [None] * NPAIR
                kt = [None] * NPAIR
                ctx_tiles = []
                for h in range(H):
                    ch = ctx_p.tile([64, S], fp32r, tag="ctx", name="ch")
                    ctx_tiles.append(ch)

                def make_qk(pr, w_t, tag):
                    dst = qk_p.tile([P, S], fp32r, tag=tag)
                    for nh in range(NH):
                        ps = pp_mm.tile([P, 512], fp32, tag="mm")
                        for dt in range(NDT):
                            mm(
                                ps,
                                w_t[:, dt, pr, :],
                                xt[dt][:, nh * 512 : (nh + 1) * 512],
                                start=(dt == 0),
                                stop=(dt == NDT - 1),
                            )
                        nc.vector.tensor_copy(dst[:, nh * 512 : (nh + 1) * 512], ps)
                    return dst

                qt[0] = make_qk(0, wq_t, "qt")
                kt[0] = make_qk(0, wk_t, "kt")
                for pr in range(NPAIR):
                    if pr + 1 < NPAIR:
                        # emit next pair's QKV before this pair's attention so
                        # its DVE evicts sit ahead of the attention DVE burst
                        qt[pr + 1] = make_qk(pr + 1, wq_t, "qt")
                        kt[pr + 1] = make_qk(pr + 1, wk_t, "kt")

                    hA, hB = 2 * pr, 2 * pr + 1
                    for nh in range(NH):
                        ssl = slice(nh * 512, (nh + 1) * 512)
                        psA = pp_acc.tile([65, 512], fp32, tag="acc")
                        psB = pp_acc.tile([65, 512], fp32, tag="acc")
                        for t in range(NST):
                            tsl = slice(t * P, (t + 1) * P)
                            scA = pp_sc.tile([P, 512], fp32, tag="sc")
                            mm(scA, kt[pr][0:64, tsl], qt[pr][0:64, ssl])
                            eA = exp_p.tile([P, 512], fp32r, tag="exp")
                            nc.scalar.activation(
                                eA, scA, AF.Exp, bias=zero_col, scale=float(SCALE)
                            )
                            mm(
                                psA,
                                v_tiles[t][:, hA, :],
                                eA,
                                start=(t == 0),
                                stop=(t == NST - 1),
                            )
                            scB = pp_sc.tile([P, 512], fp32, tag="sc")
                            mm(scB, kt[pr][64:128, tsl], qt[pr][64:128, ssl])
                            eB = exp_p.tile([P, 512], fp32r, tag="exp")
                            nc.scalar.activation(
                                eB, scB, AF.Exp, bias=zero_col, scale=float(SCALE)
                            )
                            mm(
                                psB,
                                v_tiles[t][:, hB, :],
                                eB,
                                start=(t == 0),
                                stop=(t == NST - 1),
                            )
                        # normalize: ctx rows 0-63, denom at row 64 (both heads)
                        for hh, psX in ((hA, psA), (hB, psB)):
                            rX = rows_p.tile(
                                [65, 512], fp32r, tag=f"r{hh % 2}", name="rX"
                            )
                            nc.vector.reciprocal(rX[64:65], psX[64:65])
                            bcp = pp_sc.tile([64, 512], fp32, tag="sc", name="bcp")
                            mm(bcp, ones_full[64:65, 0:64], rX[64:65])
                            bc_sb = bcs_p.tile([64, 512], fp32, tag="bcs")
                            nc.vector.tensor_copy(bc_sb, bcp)
                            nc.vector.tensor_mul(
                                ctx_tiles[hh][:, ssl], psX[0:64], bc_sb
                            )

                # ---------------- Wo + residual -> y (pre-LN1) --------------
                y = []
                for mt in range(NDT):
                    yt = mha_p.tile([P, S], fp32r, tag="mha")
                    y.append(yt)
                for mt in range(NDT):
                    for nh in range(NH):
                        ssl = slice(nh * 512, (nh + 1) * 512)
                        ps = pp_mm.tile([P, 512], fp32, tag="mm")
                        for h in range(H):
                            mm(
                                ps,
                                wo_t[:, h, mt, :],
                                ctx_tiles[h][:, ssl],
                                start=(h == 0),
                                stop=(h == H - 1),
                            )
                        nc.vector.tensor_add(y[mt][:, ssl], ps, f(xt[mt][:, ssl]))

                def layernorm(y, g_t, be_t):
                    # in-place LN over the partition (feature) dim of the 4
                    # feature-tiles in y, via ones-matmul stats + k=1 broadcast
                    for nh in range(NH):
                        ssl = slice(nh * 512, (nh + 1) * 512)
                        p1 = pp_sc.tile([1, 512], fp32, tag="sc")
                        for dt in range(NDT):
                            mm(
                                p1,
                                ones_col,
                                y[dt][:, ssl],
                                start=(dt == 0),
                                stop=(dt == NDT - 1),
                            )
                        p2 = pp_sc.tile([1, 512], fp32, tag="sc")
                        for dt in range(NDT):
                            sq = ysq_p.tile([P, 512], fp32r, tag="ysq")
                            nc.vector.tensor_mul(
                                sq, f(y[dt][:, ssl]), f(y[dt][:, ssl])
                            )
                            mm(
                                p2,
                                ones_col,
                                sq,
                                start=(dt == 0),
                                stop=(dt == NDT - 1),
                            )
                        mean = rows_p.tile([1, 512], fp32r, tag="mean", bufs=2)
                        nc.vector.tensor_scalar_mul(mean, p1, 1.0 / D)
                        msq = rows_p.tile([1, 512], fp32, tag="msq", bufs=2)
                        nc.vector.tensor_mul(msq, f(mean), f(mean))
                        var = rows_p.tile([1, 512], fp32, tag="var", bufs=2)
                        nc.vector.scalar_tensor_tensor(
                            var, p2, 1.0 / D, msq, OP.mult, OP.subtract
                        )
                        # rstd = exp(-0.5 * ln(var + eps))
                        nc.scalar.activation(var, var, AF.Ln, bias=eps_col[0:1])
                        rstd = rows_p.tile([1, 512], fp32r, tag="rstd", bufs=2)
                        nc.scalar.activation(
                            rstd, var, AF.Exp, bias=zero_col[0:1], scale=-0.5
                        )
                        bcm = pp_sc.tile([P, 512], fp32, tag="sc")
                        mm(bcm, ones_full[0:1, :], mean)
                        bcr = pp_sc.tile([P, 512], fp32, tag="sc")
                        mm(bcr, ones_full[0:1, :], rstd)
                        for dt in range(NDT):
                            nc.vector.tensor_sub(
                                y[dt][:, ssl], f(y[dt][:, ssl]), bcm
                            )
                            nc.vector.tensor_mul(
                                y[dt][:, ssl], f(y[dt][:, ssl]), bcr
                            )
                            nc.vector.tensor_scalar(
                                out=y[dt][:, ssl],
                                in0=f(y[dt][:, ssl]),
                                scalar1=g_t[:, dt : dt + 1],
                                scalar2=be_t[:, dt : dt + 1],
                                op0=OP.mult,
                                op1=OP.add,
                            )

                layernorm(y, g1_t, be1_t)  # y is now mhaT

                # ---------------- FFN ---------------------------------------
                z = []
                for mt in range(NDT):
                    zt = xt_p.tile([P, S], fp32r, tag="xt")
                    z.append(zt)
                for nh in range(NH):
                    ssl = slice(nh * 512, (nh + 1) * 512)
                    ff2_ps = [None] * NDT
                    w2_t = None
                    for fc in range(NFT // FCH):
                        w1_t = w1c_p.tile([P, NDT, FCH, P], fp32r, tag="w1c")
                        nc.sync.dma_start(out=w1_t, in_=w1_d[l, fc])
                        if fc % 2 == 0:
                            w2_t = w1c_p.tile([P, 4, NDT, P], fp32r, tag="w2c")
                            nc.sync.dma_start(out=w2_t, in_=w2_d[l, fc // 2])
                        for fi in range(FCH):
                            ft = fc * FCH + fi
                            ps = pp_mm.tile([P, 512], fp32, tag="mm")
                            for dt in range(NDT):
                                mm(
                                    ps,
                                    w1_t[:, dt, fi, :],
                                    y[dt][:, ssl],
                                    start=(dt == 0),
                                    stop=(dt == NDT - 1),
                                )
                            f1 = ff1_p.tile([P, 512], fp32r, tag="ff1")
                            nc.vector.tensor_scalar(
                                out=f1,
                                in0=ps,
                                scalar1=b1_t[:, ft : ft + 1],
                                scalar2=0.0,
                                op0=OP.add,
                                op1=OP.max,
                            )
                            # FFN2: accumulate into the 4 mt-groups
                            for mt in range(NDT):
                                if ff2_ps[mt] is None:
                                    ff2_ps[mt] = pp_acc.tile(
                                        [P, 512], fp32, tag="acc", name="ff2ps"
                                    )
                                mm(
                                    ff2_ps[mt],
                                    w2_t[:, ft % 4, mt, :],
                                    f1,
                                    start=(ft == 0),
                                    stop=(ft == NFT - 1),
                                )
                                if ft == NFT - 1:
                                    nc.vector.scalar_tensor_tensor(
                                        z[mt][:, ssl],
                                        ff2_ps[mt],
                                        b2_t[:, mt : mt + 1],
                                        f(y[mt][:, ssl]),
                                        OP.add,
                                        OP.add,
                                    )

                layernorm(z, g2_t, be2_t)  # z is now next layer's xT
                xt = z

            for dt in range(NDT):
                nc.sync.dma_start(out=out_d[dt], in_=f(xt[dt]))

    return nc


def _prep_weights(Wq, Wk, Wv, Wo, ln1_g, ln1_b, W1, b1, W2, b2, ln2_g, ln2_b):
    f = np.float32

    def qk_r(W):  # [L,H,D,DK] -> [L, 128, NDT, NPAIR, 128]
        return _round_fp32r(
            W.reshape(L, NPAIR, 2, NDT, P, DK)
            .transpose(0, 4, 3, 1, 2, 5)
            .reshape(L, P, NDT, NPAIR, P)
            .astype(f)
        )

    wv_r = _round_fp32r(
        Wv.transpose(0, 2, 1, 3)  # [L, D, H, DK]
        .reshape(L, NDT, P, H * DK)
        .transpose(0, 2, 1, 3)
        .reshape(L, P, NDT, H * DK)
        .astype(f)
    )
    # Wo packed for k=64 head-steps at partitions 0-63: [l, r, h, mt, f]
    wo_r = _round_fp32r(
        Wo.reshape(L, H, 64, NDT, P).transpose(0, 2, 1, 3, 4).astype(f)
    )
    w1_r = _round_fp32r(
        W1.reshape(L, NDT, P, NFT // FCH, FCH, P)
        .transpose(0, 3, 2, 1, 4, 5)
        .astype(f)
    )
    w2_r = _round_fp32r(
        W2.reshape(L, NFT // 4, 4, P, NDT, P).transpose(0, 1, 3, 2, 4, 5).astype(f)
    )

    def ln_r(v, n):  # [L, n*128] -> [L, 128, n]
        return np.ascontiguousarray(
            v.reshape(L, n, P).transpose(0, 2, 1).astype(f)
        )

    return {
        "wq": qk_r(Wq),
        "wk": qk_r(Wk),
        "wv": wv_r,
        "wo": wo_r,
        "w1": w1_r,
        "w2": w2_r,
        "g1": ln_r(ln1_g, NDT),
        "be1": ln_r(ln1_b, NDT),
        "g2": ln_r(ln2_g, NDT),
        "be2": ln_r(ln2_b, NDT),
        "b1": ln_r(b1, NFT),
        "b2": ln_r(b2, NDT),
    }


def get_nc():
    if "nc" not in _CACHE:
        nc = _build_nc()
        if not nc.is_finalized():
            nc.finalize()
        _CACHE["nc"] = nc
    return _CACHE["nc"]


def make_in_maps(**inputs):
    inputs = {k: np.asarray(v, dtype=np.float32) for k, v in inputs.items()}
    x = inputs.pop("x")
    wmap = _prep_weights(**inputs)
    in_maps = []
    wmap["ones"] = np.ones((P, P), dtype=np.float32)
    for b in range(B):
        xt = _round_fp32r(x[b].T.reshape(NDT, P, S))
        in_maps.append({"x": xt, **wmap})
    return in_maps


def kernel(**inputs) -> np.ndarray:
    from concourse.bass_utils import run_bass_kernel_spmd

    nc = get_nc()
    in_maps = make_in_maps(**inputs)
    res = run_bass_kernel_spmd(nc, in_maps, core_ids=list(range(B)))
    out = np.empty((B, S, D), dtype=np.float32)
    for b in range(B):
        out[b] = res.results[b]["out"].reshape(D, S).T
    return out


if __name__ == "__main__":
    rng = np.random.default_rng(0)
    ins = {
        "x": rng.standard_normal((B, S, D), dtype=np.float32),
        "Wq": rng.standard_normal((L, H, D, DK), dtype=np.float32) * 0.02,
        "Wk": rng.standard_normal((L, H, D, DK), dtype=np.float32) * 0.02,
        "Wv": rng.standard_normal((L, H, D, DK), dtype=np.float32) * 0.02,
        "Wo": rng.standard_normal((L, D, D), dtype=np.float32) * 0.02,
        "ln1_g": np.ones((L, D), np.float32),
        "ln1_b": np.zeros((L, D), np.float32),
        "W1": rng.standard_normal((L, D, DFF), dtype=np.float32) * 0.02,
        "b1": np.zeros((L, DFF), np.float32),
        "W2": rng.standard_normal((L, DFF, D), dtype=np.float32) * 0.02,
        "b2": np.zeros((L, D), np.float32),
        "ln2_g": np.ones((L, D), np.float32),
        "ln2_b": np.zeros((L, D), np.float32),
    }
    out = kernel(**ins)
    print(out.shape, out.dtype, np.abs(out).mean())



# revision 8
# speedup vs baseline: 1.0520x; 1.0520x over previous
"""Trainium2 Bass kernel for a 6-layer post-LN transformer encoder.

Sharding: data-parallel over batch — B=8, one batch element per NeuronCore,
no collectives.  Each core runs the full 6-layer encoder on its [S, D] slice.

Device-side layout: activations are kept feature-major ([D, S], "xT") in SBUF
so that every matmul can use the natural input-major weights as the stationary
(lhsT) operand and PE contracts over the partition dim:

  out[m, n] = sum_k lhsT[k, m] * rhs[k, n]

Attention is computed transposed (scoresT[t, s]) so softmax needs no
transposes: denominators fall out of a ones-column in the ctx matmul, and the
per-column 1/denom broadcast is a k=1 matmul on PE.

v2 changes vs baseline:
  * bf16 activations + weights (fp32 PSUM accumulate).  2x DVE throughput on
    SBUF elementwise ops, half the DMA traffic.  Small stats rows stay fp32r.
  * Score matmuls for the two heads of a pair are emitted adjacently: K=64
    row-group tiling (auto tile_position (0,0)/(64,0)) runs them concurrently.
  * Both heads' scores land in one [P,1024] PSUM tile -> a single Exp
    activation per (pair, nh, t) halves ACT instruction overhead.
  * ctx for head B uses an M=128 stationary [ones|0(63)|V_B] so its rows land
    at partitions 64..127 (denominator at row 0): the per-pair ctx tile is
    [128, S] and Wo contracts K=128 (half the Wo matmuls).
  * Software-pipelined emission: scores(t+1) ahead of ctx(t), FFN1(ft) ahead
    of FFN2(ft-1), QK of pair p+1 between the two nh halves of pair p.
  * relu / PSUM->SBUF broadcast copies split between ScalarE and DVE.
"""

import numpy as np

L, H, D, DK, DFF = 6, 8, 512, 64, 2048
B, S = 8, 1024
EPS = 1e-5
P = 128
NDT = D // P        # 4  d-tiles
NST = S // P        # 8  s/t-tiles
NFT = DFF // P      # 16 dff-tiles
NPAIR = H // 2      # 4  head pairs
NH = S // 512       # 2  n-halves (512-wide fp32 matmul free dim)
FCH = 2             # W1 streamed in chunks of 2 dff-tiles
VBLK = 196          # per-pair V block: [V_A(64)|1|.|1|zeros|V_B(64) @130]
SCALE = 1.0 / np.sqrt(np.float32(DK))

_CACHE = {}


def _bf16():
    from concourse import mybir

    return mybir.dt.np(mybir.dt.bfloat16)


def _build_nc():
    import concourse.bass as bass
    import concourse.bacc as bacc
    import concourse.tile as tile
    from concourse import mybir

    fp32 = mybir.dt.float32
    fp32r = mybir.dt.float32r
    bf16 = mybir.dt.bfloat16
    AF = mybir.ActivationFunctionType
    OP = mybir.AluOpType

    class _Bacc(bacc.Bacc):
        # Exp (softmax) and Ln (layernorm rstd) live in different default
        # activation-table sets, causing ~50 table-load thrashes (~2.7us
        # each). Restrict both to natural_log_exp_and_others (which holds
        # both) so one load serves the whole kernel. Positional set ids are
        # preserved; only the function->set resolution changes.
        def insert_act_table_loads(self):
            from concourse.hw_specs import get_activation_tables
            import bass_rust as _bass_rust

            has_act = any(
                isinstance(i, mybir.InstActivation)
                for b in self.main_func.blocks
                for i in b.instructions
            )
            if not has_act:
                return
            AF2 = mybir.ActivationFunctionType
            tables = []
            for name, fns in get_activation_tables(self.m.arch).items():
                if name != "natural_log_exp_and_others":
                    fns = fns - {AF2.Exp, AF2.Ln}
                tables.append((name, fns))
            _bass_rust.insert_act_table_loads(self, tables)

    nc = _Bacc()

    from concourse.hw_specs import get_activation_tables

    _nl_set = get_activation_tables(nc.m.arch).get(
        "natural_log_exp_and_others", set()
    )
    relu_on_act = AF.Relu in _nl_set and AF.Copy in _nl_set
    copy_on_act = AF.Copy in _nl_set

    def mm(out, lhsT, rhs, **kw):
        return nc.tensor.matmul(out, lhsT, rhs, **kw)

    def mmr(out, lhsT, rhs, **kw):
        # fp32r matmul for the small stats/broadcast rows
        return nc.tensor.matmul(
            out, lhsT.bitcast(fp32r), rhs.bitcast(fp32r), **kw
        )

    def f(ap):
        # view a float32r tile as plain fp32 for DVE/ACT reads
        return ap.bitcast(fp32)

    x_d = nc.declare_dram_parameter("x", [NDT, P, S], bf16, isOutput=False)
    wq_d = nc.declare_dram_parameter(
        "wq", [L, P, NDT, NPAIR, P], bf16, isOutput=False
    )
    wk_d = nc.declare_dram_parameter(
        "wk", [L, P, NDT, NPAIR, P], bf16, isOutput=False
    )
    wv_d = nc.declare_dram_parameter("wv", [L, P, NDT, H * DK], bf16, isOutput=False)
    wo_d = nc.declare_dram_parameter(
        "wo", [L, P, NPAIR, NDT, P], bf16, isOutput=False
    )
    w1_d = nc.declare_dram_parameter(
        "w1", [L, NFT // FCH, P, NDT, FCH, P], bf16, isOutput=False
    )
    w2_d = nc.declare_dram_parameter(
        "w2", [L, NFT // 4, P, 4, NDT, P], bf16, isOutput=False
    )
    g1_d = nc.declare_dram_parameter("g1", [L, P, NDT], fp32, isOutput=False)
    be1_d = nc.declare_dram_parameter("be1", [L, P, NDT], fp32, isOutput=False)
    g2_d = nc.declare_dram_parameter("g2", [L, P, NDT], fp32, isOutput=False)
    be2_d = nc.declare_dram_parameter("be2", [L, P, NDT], fp32, isOutput=False)
    b1_d = nc.declare_dram_parameter("b1", [L, P, NFT], fp32, isOutput=False)
    b2_d = nc.declare_dram_parameter("b2", [L, P, NDT], fp32, isOutput=False)
    ones_d = nc.declare_dram_parameter("ones", [P, P], fp32r, isOutput=False)
    out_d = nc.declare_dram_parameter("out", [NDT, P, S], fp32, isOutput=True)

    with tile.TileContext(nc) as tc:
        from contextlib import ExitStack

        with ExitStack() as ctx:
            ec = ctx.enter_context
            ec(
                nc.allow_low_precision(
                    reason="bf16 matmul operands; fp32 PSUM accumulation"
                )
            )
            # --- SBUF pools ---
            const_p = ec(tc.tile_pool(name="const", bufs=1))
            wts_p = ec(tc.tile_pool(name="wts", bufs=2))
            w1c_p = ec(tc.tile_pool(name="w1c", bufs=2))
            xt_p = ec(tc.tile_pool(name="xt", bufs=4))
            qk_p = ec(tc.tile_pool(name="qk", bufs=2))
            v_p = ec(tc.tile_pool(name="v", bufs=8))
            exp_p = ec(tc.tile_pool(name="exp", bufs=4))
            ctx_p = ec(tc.tile_pool(name="ctxp", bufs=4))
            mha_p = ec(tc.tile_pool(name="mha", bufs=4))
            ff1_p = ec(tc.tile_pool(name="ff1", bufs=4))
            ysq_p = ec(tc.tile_pool(name="ysq", bufs=2))
            bcs_p = ec(tc.tile_pool(name="bcs", bufs=2))
            rows_p = ec(tc.tile_pool(name="rows", bufs=1))
            out_p = ec(tc.tile_pool(name="outp", bufs=4))
            # --- PSUM pools: 4 + 2 + 2 = 8 banks ---
            # sc:  [P,1024] scores (A|B) tiles; FFN borrows [P,512] slots
            # acc: attention ctx accumulators (A, B)
            # mm:  short-lived matmul outputs (QKV/V/Wo/FFN1/LN stats+bc)
            pp_sc = ec(tc.tile_pool(name="pp_sc", bufs=2, space="PSUM"))
            pp_acc = ec(tc.tile_pool(name="pp_acc", bufs=2, space="PSUM"))
            pp_mm = ec(tc.tile_pool(name="pp_mm", bufs=2, space="PSUM"))

            # ones come from DRAM so the fp32r data counts as pre-rounded
            ones_full = const_p.tile([P, P], fp32r)
            nc.sync.dma_start(out=ones_full, in_=ones_d[:, :])
            ones_col_bf = const_p.tile([P, 1], bf16)
            nc.vector.memset(ones_col_bf, 1.0)
            # selector rows for the paired 1/denom broadcast:
            #   selA row: [1]*64 + [0]*64   selB row: [0]*64 + [1]*64
            selA_t = const_p.tile([P, P], fp32r)
            nc.vector.memset(f(selA_t)[:, 0:64], 1.0)
            nc.vector.memset(f(selA_t)[:, 64:128], 0.0)
            selB_t = const_p.tile([P, P], fp32r)
            nc.vector.memset(f(selB_t)[:, 0:64], 0.0)
            nc.vector.memset(f(selB_t)[:, 64:128], 1.0)
            zero_col = const_p.tile([P, 1], fp32)
            nc.vector.memset(zero_col, 0.0)
            eps_col = const_p.tile([P, 1], fp32)
            nc.vector.memset(eps_col, float(EPS))

            # layer-0 input
            xt = []
            for dt in range(NDT):
                t = xt_p.tile([P, S], bf16, tag="xt")
                nc.sync.dma_start(out=t, in_=x_d[dt])
                xt.append(t)

            def make_qk(pr, w_t, tag):
                dst = qk_p.tile([P, S], bf16, tag=tag, name="qkdst")
                for nh in range(NH):
                    ps = pp_mm.tile([P, 512], fp32, tag="mm", name="qkps")
                    for dt in range(NDT):
                        mm(
                            ps,
                            w_t[:, dt, pr, :],
                            xt[dt][:, nh * 512 : (nh + 1) * 512],
                            start=(dt == 0),
                            stop=(dt == NDT - 1),
                        )
                    nc.vector.tensor_copy(dst[:, nh * 512 : (nh + 1) * 512], ps)
                return dst

            for l in range(L):
                # ---------------- weight loads (bufs=2 pools: next layer's
                # loads overlap this layer's compute) ------------------------
                wq_t = wts_p.tile([P, NDT, NPAIR, P], bf16, tag="wq")
                nc.sync.dma_start(out=wq_t, in_=wq_d[l])
                wk_t = wts_p.tile([P, NDT, NPAIR, P], bf16, tag="wk")
                nc.sync.dma_start(out=wk_t, in_=wk_d[l])
                wv_t = wts_p.tile([P, NDT, H * DK], bf16, tag="wv")
                nc.sync.dma_start(out=wv_t, in_=wv_d[l])
                wo_t = wts_p.tile([P, NPAIR, NDT, P], bf16, tag="wo")
                nc.sync.dma_start(out=wo_t, in_=wo_d[l])
                g1_t = wts_p.tile([P, NDT], fp32, tag="g1")
                nc.sync.dma_start(out=g1_t, in_=g1_d[l])
                be1_t = wts_p.tile([P, NDT], fp32, tag="be1")
                nc.sync.dma_start(out=be1_t, in_=be1_d[l])
                g2_t = wts_p.tile([P, NDT], fp32, tag="g2")
                nc.sync.dma_start(out=g2_t, in_=g2_d[l])
                be2_t = wts_p.tile([P, NDT], fp32, tag="be2")
                nc.sync.dma_start(out=be2_t, in_=be2_d[l])
                b1_t = wts_p.tile([P, NFT], fp32, tag="b1")
                nc.sync.dma_start(out=b1_t, in_=b1_d[l])
                b2_t = wts_p.tile([P, NDT], fp32, tag="b2")
                nc.sync.dma_start(out=b2_t, in_=b2_d[l])

                # ---------------- Q/K for pair 0 ----------------------------
                qt = [None] * NPAIR
                kt = [None] * NPAIR
                qt[0] = make_qk(0, wq_t, "qt")
                kt[0] = make_qk(0, wk_t, "kt")

                # ---------------- V = x @ Wv, packed per head pair ----------
                # vt[:, pr, 0:64]    = V of head 2*pr       (ctx rows 0..63)
                # vt[:, pr, 64]      = 1                    (denom A, row 64)
                # vt[:, pr, 66]      = 1                    (denom B, row 0)
                # vt[:, pr, 67:130]  = 0                    (junk rows 1..63)
                # vt[:, pr, 130:194] = V of head 2*pr+1     (ctx rows 64..127)
                v_tiles = []
                for st in range(NST):
                    vt = v_p.tile([P, NPAIR, VBLK], bf16, tag="v")
                    ps = pp_mm.tile([P, 512], fp32, tag="mm", name="vps")
                    for dt in range(NDT):
                        mm(
                            ps,
                            xt[dt][:, st * P : (st + 1) * P],
                            wv_t[:, dt, :],
                            start=(dt == 0),
                            stop=(dt == NDT - 1),
                        )
                    psh = ps.rearrange("p (h k) -> p h k", h=H)
                    nc.vector.tensor_copy(vt[:, :, 0:DK], psh[:, 0::2, :])
                    nc.vector.tensor_copy(vt[:, :, 130 : 130 + DK], psh[:, 1::2, :])
                    nc.gpsimd.memset(vt[:, :, 67:130], 0.0)
                    nc.gpsimd.memset(vt[:, :, 64:65], 1.0)
                    nc.gpsimd.memset(vt[:, :, 66:67], 1.0)
                    v_tiles.append(vt)

                # ---------------- attention ---------------------------------
                ctx_tiles = []
                for pr in range(NPAIR):
                    ch = ctx_p.tile([P, S], bf16, tag="ctx", name="ch")
                    ctx_tiles.append(ch)

                def scores(pr, nh, t):
                    ssl = slice(nh * 512, (nh + 1) * 512)
                    tsl = slice(t * P, (t + 1) * P)
                    sc = pp_sc.tile([P, 1024], fp32, tag="sc", name="sc")
                    # two K=64 matmuls on distinct row groups -> concurrent
                    mm(sc[:, 0:512], kt[pr][0:64, tsl], qt[pr][0:64, ssl])
                    mm(sc[:, 512:1024], kt[pr][64:128, tsl], qt[pr][64:128, ssl])
                    return sc

                def attend(pr, nh):
                    ssl = slice(nh * 512, (nh + 1) * 512)
                    psA = pp_acc.tile([P, 512], fp32, tag="acc", name="psA")
                    psB = pp_acc.tile([P, 512], fp32, tag="acc", name="psB")
                    sc_cur = scores(pr, nh, 0)
                    for t in range(NST):
                        sc_next = scores(pr, nh, t + 1) if t + 1 < NST else None
                        e = exp_p.tile([P, 1024], bf16, tag="exp", name="e")
                        nc.scalar.activation(
                            e, sc_cur, AF.Exp, bias=zero_col, scale=float(SCALE)
                        )
                        vt = v_tiles[t]
                        mm(
                            psA[0:65, :],
                            vt[:, pr, 0 : DK + 1],
                            e[:, 0:512],
                            start=(t == 0),
                            stop=(t == NST - 1),
                        )
                        mm(
                            psB,
                            vt[:, pr, 66:194],
                            e[:, 512:1024],
                            start=(t == 0),
                            stop=(t == NST - 1),
                        )
                        sc_cur = sc_next
                    # normalize: ctx rows / denominator (A: row 64, B: row 0)
                    rA = rows_p.tile([65, 512], fp32r, tag="rA", bufs=2, name="rA")
                    nc.vector.reciprocal(rA[64:65], psA[64:65])
                    rB = rows_p.tile([1, 512], fp32r, tag="rB", bufs=2, name="rB")
                    nc.vector.reciprocal(rB, psB[0:1])
                    # bc rows 0..63 = 1/dA, rows 64..127 = 1/dB via selector
                    # rows (two accumulating M=128 matmuls, no col tiling)
                    bc = pp_mm.tile([P, 512], fp32, tag="mm", name="bc")
                    mmr(bc, selA_t[64:65, :], rA[64:65], start=True, stop=False)
                    mmr(bc, selB_t[0:1, :], rB, start=False, stop=True)
                    bc_sb = bcs_p.tile([P, 512], bf16, tag="bcs", name="bcsb")
                    if copy_on_act:
                        nc.scalar.copy(bc_sb, bc)
                    else:
                        nc.vector.tensor_copy(bc_sb, bc)
                    ch = ctx_tiles[pr]
                    nc.vector.tensor_mul(ch[0:64, ssl], psA[0:64], bc_sb[0:64])
                    nc.vector.tensor_mul(ch[64:128, ssl], psB[64:128], bc_sb[64:128])

                for pr in range(NPAIR):
                    attend(pr, 0)
                    if pr + 1 < NPAIR:
                        # PE work to cover the normalize tail / acc release
                        qt[pr + 1] = make_qk(pr + 1, wq_t, "qt")
                        kt[pr + 1] = make_qk(pr + 1, wk_t, "kt")
                    attend(pr, 1)

                # ---------------- Wo + residual -> y (pre-LN1) --------------
                y = []
                for mt in range(NDT):
                    yt = mha_p.tile([P, S], bf16, tag="mha", name="yt")
                    y.append(yt)
                for mt in range(NDT):
                    for nh in range(NH):
                        ssl = slice(nh * 512, (nh + 1) * 512)
                        ps = pp_mm.tile([P, 512], fp32, tag="mm", name="wops")
                        for pr in range(NPAIR):
                            mm(
                                ps,
                                wo_t[:, pr, mt, :],
                                ctx_tiles[pr][:, ssl],
                                start=(pr == 0),
                                stop=(pr == NPAIR - 1),
                            )
                        nc.vector.tensor_add(y[mt][:, ssl], ps, xt[mt][:, ssl])

                def layernorm(yl, g_t, be_t):
                    # in-place LN over the partition (feature) dim of the 4
                    # feature-tiles in yl, via ones-matmul stats + k=1 bcast
                    for nh in range(NH):
                        ssl = slice(nh * 512, (nh + 1) * 512)
                        p1 = pp_mm.tile([1, 512], fp32, tag="mm", name="p1")
                        for dt in range(NDT):
                            mm(
                                p1,
                                ones_col_bf,
                                yl[dt][:, ssl],
                                start=(dt == 0),
                                stop=(dt == NDT - 1),
                            )
                        p2 = pp_mm.tile([1, 512], fp32, tag="mm", name="p2")
                        for dt in range(NDT):
                            sq = ysq_p.tile([P, 512], bf16, tag="ysq", name="sq")
                            nc.vector.tensor_mul(sq, yl[dt][:, ssl], yl[dt][:, ssl])
                            mm(
                                p2,
                                ones_col_bf,
                                sq,
                                start=(dt == 0),
                                stop=(dt == NDT - 1),
                            )
                        mean = rows_p.tile([1, 512], fp32r, tag="mean", bufs=2)
                        nc.vector.tensor_scalar_mul(mean, p1, 1.0 / D)
                        msq = rows_p.tile([1, 512], fp32, tag="msq", bufs=2)
                        nc.vector.tensor_mul(msq, f(mean), f(mean))
                        var = rows_p.tile([1, 512], fp32, tag="var", bufs=2)
                        nc.vector.scalar_tensor_tensor(
                            var, p2, 1.0 / D, msq, OP.mult, OP.subtract
                        )
                        # rstd = exp(-0.5 * ln(var + eps))
                        nc.scalar.activation(var, var, AF.Ln, bias=eps_col[0:1])
                        rstd = rows_p.tile([1, 512], fp32r, tag="rstd", bufs=2)
                        nc.scalar.activation(
                            rstd, var, AF.Exp, bias=zero_col[0:1], scale=-0.5
                        )
                        bcm = pp_mm.tile([P, 512], fp32, tag="mm", name="bcm")
                        mmr(bcm, ones_full[0:1, :], mean)
                        bcr = pp_mm.tile([P, 512], fp32, tag="mm", name="bcr")
                        mmr(bcr, ones_full[0:1, :], rstd)
                        bcm_sb = bcs_p.tile([P, 512], bf16, tag="bcs", name="bcmsb")
                        bcr_sb = bcs_p.tile([P, 512], bf16, tag="bcs", name="bcrsb")
                        if copy_on_act:
                            nc.scalar.copy(bcm_sb, bcm)
                            nc.scalar.copy(bcr_sb, bcr)
                        else:
                            nc.vector.tensor_copy(bcm_sb, bcm)
                            nc.vector.tensor_copy(bcr_sb, bcr)
                        for dt in range(NDT):
                            ysl = yl[dt][:, ssl]
                            nc.vector.tensor_sub(ysl, ysl, bcm_sb)
                            nc.vector.tensor_mul(ysl, ysl, bcr_sb)
                            nc.vector.tensor_scalar(
                                out=ysl,
                                in0=ysl,
                                scalar1=g_t[:, dt : dt + 1],
                                scalar2=be_t[:, dt : dt + 1],
                                op0=OP.mult,
                                op1=OP.add,
                            )

                layernorm(y, g1_t, be1_t)  # y is now mhaT

                # ---------------- FFN ---------------------------------------
                z = []
                for mt in range(NDT):
                    zt = xt_p.tile([P, S], bf16, tag="xt", name="zt",
                                   padded_shape=[P, 2 * S])
                    z.append(zt)
                for nh in range(NH):
                    ssl = slice(nh * 512, (nh + 1) * 512)
                    ff2_ps = []
                    for mt in range(NDT):
                        pool = pp_acc if mt < 2 else pp_sc
                        tag = "acc" if mt < 2 else "sc"
                        ff2_ps.append(
                            pool.tile([P, 512], fp32, tag=tag, name="ff2ps")
                        )
                    pending = None  # (ft, f1 tile) awaiting FFN2 emission
                    w2_t = None
                    for fc in range(NFT // FCH):
                        w1_t = w1c_p.tile([P, NDT, FCH, P], bf16, tag="w1c")
                        nc.sync.dma_start(out=w1_t, in_=w1_d[l, fc])
                        if fc % 2 == 0:
                            w2_t = w1c_p.tile([P, 4, NDT, P], bf16, tag="w2c")
                            nc.sync.dma_start(out=w2_t, in_=w2_d[l, fc // 2])
                        w2_cur = w2_t
                        for fi in range(FCH):
                            ft = fc * FCH + fi
                            ps = pp_mm.tile([P, 512], fp32, tag="mm", name="f1ps")
                            for dt in range(NDT):
                                mm(
                                    ps,
                                    w1_t[:, dt, fi, :],
                                    y[dt][:, ssl],
                                    start=(dt == 0),
                                    stop=(dt == NDT - 1),
                                )
                            f1 = ff1_p.tile([P, 512], bf16, tag="ff1", name="f1")
                            if relu_on_act and ft % 2 == 0:
                                nc.scalar.activation(
                                    f1, ps, AF.Relu, bias=b1_t[:, ft : ft + 1]
                                )
                            else:
                                nc.vector.tensor_scalar(
                                    out=f1,
                                    in0=ps,
                                    scalar1=b1_t[:, ft : ft + 1],
                                    scalar2=0.0,
                                    op0=OP.add,
                                    op1=OP.max,
                                )
                            if pending is not None:
                                pft, pf1, pw2 = pending
                                for mt in range(NDT):
                                    mm(
                                        ff2_ps[mt],
                                        pw2[:, pft % 4, mt, :],
                                        pf1,
                                        start=(pft == 0),
                                        stop=False,
                                    )
                            pending = (ft, f1, w2_cur)
                    pft, pf1, pw2 = pending
                    for mt in range(NDT):
                        mm(
                            ff2_ps[mt],
                            pw2[:, pft % 4, mt, :],
                            pf1,
                            start=False,
                            stop=True,
                        )
                        nc.vector.scalar_tensor_tensor(
                            z[mt][:, ssl],
                            ff2_ps[mt],
                            b2_t[:, mt : mt + 1],
                            y[mt][:, ssl],
                            OP.add,
                            OP.add,
                        )

                layernorm(z, g2_t, be2_t)  # z is now next layer's xT
                xt = z

            for dt in range(NDT):
                ot = out_p.tile([P, S], fp32, tag="out", name="ot")
                if copy_on_act:
                    nc.scalar.copy(ot, xt[dt])
                else:
                    nc.vector.tensor_copy(ot, xt[dt])
                nc.sync.dma_start(out=out_d[dt], in_=ot)

    return nc


def _prep_weights(Wq, Wk, Wv, Wo, ln1_g, ln1_b, W1, b1, W2, b2, ln2_g, ln2_b):
    f = np.float32
    bf = _bf16()

    def qk_r(W):  # [L,H,D,DK] -> [L, 128, NDT, NPAIR, 128]
        return np.ascontiguousarray(
            W.reshape(L, NPAIR, 2, NDT, P, DK)
            .transpose(0, 4, 3, 1, 2, 5)
            .reshape(L, P, NDT, NPAIR, P)
            .astype(bf)
        )

    wv_r = np.ascontiguousarray(
        Wv.transpose(0, 2, 1, 3)  # [L, D, H, DK]
        .reshape(L, NDT, P, H * DK)
        .transpose(0, 2, 1, 3)
        .reshape(L, P, NDT, H * DK)
        .astype(bf)
    )
    # Wo packed for K=128 pair-steps: [l, (head01, dk)=128, pr, mt, f]
    wo_r = np.ascontiguousarray(
        Wo.reshape(L, NPAIR, P, NDT, P).transpose(0, 2, 1, 3, 4).astype(bf)
    )
    w1_r = np.ascontiguousarray(
        W1.reshape(L, NDT, P, NFT // FCH, FCH, P)
        .transpose(0, 3, 2, 1, 4, 5)
        .astype(bf)
    )
    w2_r = np.ascontiguousarray(
        W2.reshape(L, NFT // 4, 4, P, NDT, P).transpose(0, 1, 3, 2, 4, 5).astype(bf)
    )

    def ln_r(v, n):  # [L, n*128] -> [L, 128, n]
        return np.ascontiguousarray(
            v.reshape(L, n, P).transpose(0, 2, 1).astype(f)
        )

    return {
        "wq": qk_r(Wq),
        "wk": qk_r(Wk),
        "wv": wv_r,
        "wo": wo_r,
        "w1": w1_r,
        "w2": w2_r,
        "g1": ln_r(ln1_g, NDT),
        "be1": ln_r(ln1_b, NDT),
        "g2": ln_r(ln2_g, NDT),
        "be2": ln_r(ln2_b, NDT),
        "b1": ln_r(b1, NFT),
        "b2": ln_r(b2, NDT),
    }


def get_nc():
    if "nc" not in _CACHE:
        nc = _build_nc()
        if not nc.is_finalized():
            nc.finalize()
        _CACHE["nc"] = nc
    return _CACHE["nc"]


def make_in_maps(**inputs):
    inputs = {k: np.asarray(v, dtype=np.float32) for k, v in inputs.items()}
    x = inputs.pop("x")
    wmap = _prep_weights(**inputs)
    in_maps = []
    wmap["ones"] = np.ones((P, P), dtype=np.float32)
    bf = _bf16()
    for b in range(B):
        xt = np.ascontiguousarray(x[b].T.reshape(NDT, P, S).astype(bf))
        in_maps.append({"x": xt, **wmap})
    return in_maps


def kernel(**inputs) -> np.ndarray:
    from concourse.bass_utils import run_bass_kernel_spmd

    nc = get_nc()
    in_maps = make_in_maps(**inputs)
    res = run_bass_kernel_spmd(nc, in_maps, core_ids=list(range(B)))
    out = np.empty((B, S, D), dtype=np.float32)
    for b in range(B):
        out[b] = res.results[b]["out"].reshape(D, S).T
    return out


if __name__ == "__main__":
    rng = np.random.default_rng(0)
    ins = {
        "x": rng.standard_normal((B, S, D), dtype=np.float32),
        "Wq": rng.standard_normal((L, H, D, DK), dtype=np.float32) * 0.02,
        "Wk": rng.standard_normal((L, H, D, DK), dtype=np.float32) * 0.02,
        "Wv": rng.standard_normal((L, H, D, DK), dtype=np.float32) * 0.02,
        "Wo": rng.standard_normal((L, D, D), dtype=np.float32) * 0.02,
        "ln1_g": np.ones((L, D), np.float32),
        "ln1_b": np.zeros((L, D), np.float32),
        "W1": rng.standard_normal((L, D, DFF), dtype=np.float32) * 0.02,
        "b1": np.zeros((L, DFF), np.float32),
        "W2": rng.standard_normal((L, DFF, D), dtype=np.float32) * 0.02,
        "b2": np.zeros((L, D), np.float32),
        "ln2_g": np.ones((L, D), np.float32),
        "ln2_b": np.zeros((L, D), np.float32),
    }
    out = kernel(**ins)
    print(out.shape, out.dtype, np.abs(out).mean())


# revision 23
# speedup vs baseline: 1.6238x; 1.5435x over previous
"""Trainium2 Bass kernel for a 6-layer post-LN transformer encoder.

Sharding: data-parallel over batch — B=8, one batch element per NeuronCore,
no collectives.  Each core runs the full 6-layer encoder on its [S, D] slice.

Device-side layout: activations are kept feature-major ([D, S], "xT") in SBUF
so that every matmul can use the natural input-major weights as the stationary
(lhsT) operand and PE contracts over the partition dim:

  out[m, n] = sum_k lhsT[k, m] * rhs[k, n]

Attention is computed transposed (scoresT[t, s]) so softmax needs no
transposes: denominators fall out of a ones-column in the ctx matmul, and the
per-column 1/denom broadcast is a k=1 matmul on PE.

v2 changes vs baseline:
  * bf16 activations + weights (fp32 PSUM accumulate).  2x DVE throughput on
    SBUF elementwise ops, half the DMA traffic.  Small stats rows stay fp32r.
  * Score matmuls for the two heads of a pair are emitted adjacently: K=64
    row-group tiling (auto tile_position (0,0)/(64,0)) runs them concurrently.
  * Both heads' scores land in one [P,1024] PSUM tile -> a single Exp
    activation per (pair, nh, t) halves ACT instruction overhead.
  * ctx for head B uses an M=128 stationary [ones|0(63)|V_B] so its rows land
    at partitions 64..127 (denominator at row 0): the per-pair ctx tile is
    [128, S] and Wo contracts K=128 (half the Wo matmuls).
  * Software-pipelined emission: scores(t+1) ahead of ctx(t), FFN1(ft) ahead
    of FFN2(ft-1), QK of pair p+1 between the two nh halves of pair p.
  * relu / PSUM->SBUF broadcast copies split between ScalarE and DVE.
"""

import numpy as np

L, H, D, DK, DFF = 6, 8, 512, 64, 2048
B, S = 8, 1024
EPS = 1e-5
P = 128
NDT = D // P        # 4  d-tiles
NST = S // P        # 8  s/t-tiles
NFT = DFF // P      # 16 dff-tiles
NPAIR = H // 2      # 4  head pairs
NH = S // 512       # 2  n-halves (512-wide fp32 matmul free dim)
FCH = 2             # W1 streamed in chunks of 2 dff-tiles
VBLK = 196          # per-pair V block: [V_A(64)|1|.|1|zeros|V_B(64) @130]
SCALE = 1.0 / np.sqrt(np.float32(DK))

_CACHE = {}


def _bf16():
    from concourse import mybir

    return mybir.dt.np(mybir.dt.bfloat16)


def _build_nc():
    import concourse.bass as bass
    import concourse.bacc as bacc
    import concourse.tile as tile
    from concourse import mybir

    fp32 = mybir.dt.float32
    fp32r = mybir.dt.float32r
    bf16 = mybir.dt.bfloat16
    AF = mybir.ActivationFunctionType
    OP = mybir.AluOpType

    class _Bacc(bacc.Bacc):
        # Exp (softmax) and Ln (layernorm rstd) live in different default
        # activation-table sets, causing ~50 table-load thrashes (~2.7us
        # each). Restrict both to natural_log_exp_and_others (which holds
        # both) so one load serves the whole kernel. Positional set ids are
        # preserved; only the function->set resolution changes.
        def insert_act_table_loads(self):
            from concourse.hw_specs import get_activation_tables
            import bass_rust as _bass_rust

            has_act = any(
                isinstance(i, mybir.InstActivation)
                for b in self.main_func.blocks
                for i in b.instructions
            )
            if not has_act:
                return
            AF2 = mybir.ActivationFunctionType
            tables = []
            for name, fns in get_activation_tables(self.m.arch).items():
                if name != "natural_log_exp_and_others":
                    fns = fns - {AF2.Exp, AF2.Ln}
                tables.append((name, fns))
            _bass_rust.insert_act_table_loads(self, tables)

    nc = _Bacc()

    from concourse.hw_specs import get_activation_tables

    _nl_set = get_activation_tables(nc.m.arch).get(
        "natural_log_exp_and_others", set()
    )
    relu_on_act = AF.Relu in _nl_set and AF.Copy in _nl_set
    copy_on_act = AF.Copy in _nl_set

    def mm(out, lhsT, rhs, **kw):
        return nc.tensor.matmul(out, lhsT, rhs, **kw)

    def mmr(out, lhsT, rhs, **kw):
        # fp32r matmul for the small stats/broadcast rows
        return nc.tensor.matmul(
            out, lhsT.bitcast(fp32r), rhs.bitcast(fp32r), **kw
        )

    def f(ap):
        # view a float32r tile as plain fp32 for DVE/ACT reads
        return ap.bitcast(fp32)

    x_d = nc.declare_dram_parameter("x", [NDT, P, S], bf16, isOutput=False)
    wq_d = nc.declare_dram_parameter(
        "wq", [L, P, NDT, NPAIR, P], bf16, isOutput=False
    )
    wk_d = nc.declare_dram_parameter(
        "wk", [L, P, NDT, NPAIR, P], bf16, isOutput=False
    )
    wv_d = nc.declare_dram_parameter("wv", [L, P, NDT, H * DK], bf16, isOutput=False)
    wo_d = nc.declare_dram_parameter(
        "wo", [L, P, NPAIR, NDT, P], bf16, isOutput=False
    )
    w1_d = nc.declare_dram_parameter(
        "w1", [L, P, NDT, NFT, P], bf16, isOutput=False
    )
    w2_d = nc.declare_dram_parameter(
        "w2", [L, P, NFT, NDT, P], bf16, isOutput=False
    )
    g1_d = nc.declare_dram_parameter("g1", [L, P, NDT], fp32, isOutput=False)
    be1_d = nc.declare_dram_parameter("be1", [L, P, NDT], fp32, isOutput=False)
    g2_d = nc.declare_dram_parameter("g2", [L, P, NDT], fp32, isOutput=False)
    be2_d = nc.declare_dram_parameter("be2", [L, P, NDT], fp32, isOutput=False)
    b1_d = nc.declare_dram_parameter("b1", [L, P, NFT], fp32, isOutput=False)
    b2_d = nc.declare_dram_parameter("b2", [L, P, NDT], fp32, isOutput=False)
    ones_d = nc.declare_dram_parameter("ones", [P, P], fp32r, isOutput=False)
    out_d = nc.declare_dram_parameter("out", [NDT, P, S], fp32, isOutput=True)

    with tile.TileContext(nc) as tc:
        from contextlib import ExitStack

        with ExitStack() as ctx:
            ec = ctx.enter_context
            ec(
                nc.allow_low_precision(
                    reason="bf16 matmul operands; fp32 PSUM accumulation"
                )
            )
            # --- SBUF pools ---
            const_p = ec(tc.tile_pool(name="const", bufs=1))
            wts_p = ec(tc.tile_pool(name="wts", bufs=2))
            xt_p = ec(tc.tile_pool(name="xt", bufs=4))
            qk_p = ec(tc.tile_pool(name="qk", bufs=2))
            v_p = ec(tc.tile_pool(name="v", bufs=8))
            exp_p = ec(tc.tile_pool(name="exp", bufs=4))
            ctx_p = ec(tc.tile_pool(name="ctxp", bufs=4))
            mha_p = ec(tc.tile_pool(name="mha", bufs=4))
            ff1_p = ec(tc.tile_pool(name="ff1", bufs=4))
            ysq_p = ec(tc.tile_pool(name="ysq", bufs=2))
            bcs_p = ec(tc.tile_pool(name="bcs", bufs=2))
            rows_p = ec(tc.tile_pool(name="rows", bufs=1))
            out_p = ec(tc.tile_pool(name="outp", bufs=4))
            # --- PSUM pools: 4 + 2 + 2 = 8 banks ---
            # sc:  [P,1024] scores (A|B) tiles; FFN borrows [P,512] slots
            # acc: attention ctx accumulators (A, B)
            # mm:  short-lived matmul outputs (QKV/V/Wo/FFN1/LN stats+bc)
            pp_sc = ec(tc.tile_pool(name="pp_sc", bufs=2, space="PSUM"))
            pp_acc = ec(tc.tile_pool(name="pp_acc", bufs=2, space="PSUM"))
            pp_mm = ec(tc.tile_pool(name="pp_mm", bufs=2, space="PSUM"))

            # ones come from DRAM so the fp32r data counts as pre-rounded
            ones_full = const_p.tile([P, P], fp32r)
            nc.sync.dma_start(out=ones_full, in_=ones_d[:, :])
            ones_col_bf = const_p.tile([P, 1], bf16)
            nc.vector.memset(ones_col_bf, 1.0)
            # selector rows for the paired 1/denom broadcast:
            #   selA row: [1]*64 + [0]*64   selB row: [0]*64 + [1]*64
            selA_t = const_p.tile([P, P], fp32r)
            nc.vector.memset(f(selA_t)[:, 0:64], 1.0)
            nc.vector.memset(f(selA_t)[:, 64:128], 0.0)
            selB_t = const_p.tile([P, P], fp32r)
            nc.vector.memset(f(selB_t)[:, 0:64], 0.0)
            nc.vector.memset(f(selB_t)[:, 64:128], 1.0)
            zero_col = const_p.tile([P, 1], fp32)
            nc.vector.memset(zero_col, 0.0)
            eps_col = const_p.tile([P, 1], fp32)
            nc.vector.memset(eps_col, float(EPS))

            # layer-0 input
            xt = []
            for dt in range(NDT):
                t = xt_p.tile([P, S], bf16, tag="xt")
                nc.sync.dma_start(out=t, in_=x_d[dt])
                xt.append(t)

            def make_qk(pr, w_t, tag):
                dst = qk_p.tile([P, S], bf16, tag=tag, name="qkdst")
                for nh in range(NH):
                    ps = pp_mm.tile([P, 512], fp32, tag="mm", name="qkps")
                    for dt in range(NDT):
                        mm(
                            ps,
                            w_t[:, dt, pr, :],
                            xt[dt][:, nh * 512 : (nh + 1) * 512],
                            start=(dt == 0),
                            stop=(dt == NDT - 1),
                        )
                    nc.vector.tensor_copy(dst[:, nh * 512 : (nh + 1) * 512], ps)
                return dst

            for l in range(L):
                # ---------------- weight loads (bufs=2 pools: next layer's
                # loads overlap this layer's compute) ------------------------
                wq_t = wts_p.tile([P, NDT, NPAIR, P], bf16, tag="wq")
                nc.sync.dma_start(out=wq_t, in_=wq_d[l])
                wk_t = wts_p.tile([P, NDT, NPAIR, P], bf16, tag="wk")
                nc.sync.dma_start(out=wk_t, in_=wk_d[l])
                wv_t = wts_p.tile([P, NDT, H * DK], bf16, tag="wv")
                nc.sync.dma_start(out=wv_t, in_=wv_d[l])
                wo_t = wts_p.tile([P, NPAIR, NDT, P], bf16, tag="wo")
                nc.sync.dma_start(out=wo_t, in_=wo_d[l])
                g1_t = wts_p.tile([P, NDT], fp32, tag="g1")
                nc.sync.dma_start(out=g1_t, in_=g1_d[l])
                be1_t = wts_p.tile([P, NDT], fp32, tag="be1")
                nc.sync.dma_start(out=be1_t, in_=be1_d[l])
                g2_t = wts_p.tile([P, NDT], fp32, tag="g2")
                nc.sync.dma_start(out=g2_t, in_=g2_d[l])
                be2_t = wts_p.tile([P, NDT], fp32, tag="be2")
                nc.sync.dma_start(out=be2_t, in_=be2_d[l])
                b1_t = wts_p.tile([P, NFT], fp32, tag="b1")
                nc.sync.dma_start(out=b1_t, in_=b1_d[l])
                b2_t = wts_p.tile([P, NDT], fp32, tag="b2")
                nc.sync.dma_start(out=b2_t, in_=b2_d[l])
                w1_t = wts_p.tile([P, NDT, NFT, P], bf16, tag="w1")
                nc.sync.dma_start(out=w1_t, in_=w1_d[l])
                w2_t = wts_p.tile([P, NFT, NDT, P], bf16, tag="w2")
                nc.sync.dma_start(out=w2_t, in_=w2_d[l])

                # ---------------- Q/K for pair 0 ----------------------------
                qt = [None] * NPAIR
                kt = [None] * NPAIR
                qt[0] = make_qk(0, wq_t, "qt")
                kt[0] = make_qk(0, wk_t, "kt")

                # ---------------- V = x @ Wv, packed per head pair ----------
                # vt[:, pr, 0:64]    = V of head 2*pr       (ctx rows 0..63)
                # vt[:, pr, 64]      = 1                    (denom A, row 64)
                # vt[:, pr, 66]      = 1                    (denom B, row 0)
                # vt[:, pr, 67:130]  = 0                    (junk rows 1..63)
                # vt[:, pr, 130:194] = V of head 2*pr+1     (ctx rows 64..127)
                v_tiles = []
                for st in range(NST):
                    vt = v_p.tile([P, NPAIR, VBLK], bf16, tag="v")
                    ps = pp_mm.tile([P, 512], fp32, tag="mm", name="vps")
                    for dt in range(NDT):
                        mm(
                            ps,
                            xt[dt][:, st * P : (st + 1) * P],
                            wv_t[:, dt, :],
                            start=(dt == 0),
                            stop=(dt == NDT - 1),
                        )
                    psh = ps.rearrange("p (h k) -> p h k", h=H)
                    nc.vector.tensor_copy(vt[:, :, 0:DK], psh[:, 0::2, :])
                    nc.vector.tensor_copy(vt[:, :, 130 : 130 + DK], psh[:, 1::2, :])
                    nc.gpsimd.memset(vt[:, :, 67:130], 0.0)
                    nc.gpsimd.memset(vt[:, :, 64:65], 1.0)
                    nc.gpsimd.memset(vt[:, :, 66:67], 1.0)
                    v_tiles.append(vt)

                # ---------------- attention ---------------------------------
                ctx_tiles = []
                for pr in range(NPAIR):
                    ch = ctx_p.tile([P, S], bf16, tag="ctx", name="ch")
                    ctx_tiles.append(ch)

                def scores(pr, nh, t):
                    ssl = slice(nh * 512, (nh + 1) * 512)
                    tsl = slice(t * P, (t + 1) * P)
                    sc = pp_sc.tile([P, 1024], fp32, tag="sc", name="sc")
                    # two K=64 matmuls on distinct row groups -> concurrent
                    mm(sc[:, 0:512], kt[pr][0:64, tsl], qt[pr][0:64, ssl])
                    mm(sc[:, 512:1024], kt[pr][64:128, tsl], qt[pr][64:128, ssl])
                    return sc

                def attend(pr, nh):
                    ssl = slice(nh * 512, (nh + 1) * 512)
                    psA = pp_acc.tile([P, 512], fp32, tag="acc", name="psA")
                    psB = pp_acc.tile([P, 512], fp32, tag="acc", name="psB")
                    sc_cur = scores(pr, nh, 0)
                    for t in range(NST):
                        sc_next = scores(pr, nh, t + 1) if t + 1 < NST else None
                        e = exp_p.tile([P, 1024], bf16, tag="exp", name="e")
                        nc.scalar.activation(
                            e, sc_cur, AF.Exp, bias=zero_col, scale=float(SCALE)
                        )
                        vt = v_tiles[t]
                        mm(
                            psA[0:65, :],
                            vt[:, pr, 0 : DK + 1],
                            e[:, 0:512],
                            start=(t == 0),
                            stop=(t == NST - 1),
                        )
                        mm(
                            psB,
                            vt[:, pr, 66:194],
                            e[:, 512:1024],
                            start=(t == 0),
                            stop=(t == NST - 1),
                        )
                        sc_cur = sc_next
                    # normalize: ctx rows / denominator (A: row 64, B: row 0).
                    # ScalarE copies the raw ctx rows out first so the acc
                    # PSUM banks release early (next pair's ctx can start);
                    # the 1/denom scale then runs in-place in SBUF with the
                    # broadcast read straight from PSUM.
                    rA = rows_p.tile([65, 512], fp32r, tag="rA", bufs=2, name="rA")
                    nc.vector.reciprocal(rA[64:65], psA[64:65])
                    rB = rows_p.tile([1, 512], fp32r, tag="rB", bufs=2, name="rB")
                    nc.vector.reciprocal(rB, psB[0:1])
                    ch = ctx_tiles[pr]
                    nc.vector.tensor_copy(ch[0:64, ssl], psA[0:64])
                    nc.vector.tensor_copy(ch[64:128, ssl], psB[64:128])
                    # bc rows 0..63 = 1/dA, rows 64..127 = 1/dB via selector
                    # rows (two accumulating M=128 matmuls, no col tiling)
                    bc = pp_mm.tile([P, 512], fp32, tag="mm", name="bc")
                    mmr(bc, selA_t[64:65, :], rA[64:65], start=True, stop=False)
                    mmr(bc, selB_t[0:1, :], rB, start=False, stop=True)
                    nc.vector.tensor_mul(ch[0:64, ssl], ch[0:64, ssl], bc[0:64])
                    nc.vector.tensor_mul(ch[64:128, ssl], ch[64:128, ssl], bc[64:128])

                for pr in range(NPAIR):
                    attend(pr, 0)
                    if pr + 1 < NPAIR:
                        # PE work to cover the normalize tail / acc release
                        qt[pr + 1] = make_qk(pr + 1, wq_t, "qt")
                        kt[pr + 1] = make_qk(pr + 1, wk_t, "kt")
                    attend(pr, 1)

                # ---------------- Wo + residual -> y (pre-LN1) --------------
                y = []
                for mt in range(NDT):
                    yt = mha_p.tile([P, S], bf16, tag="mha", name="yt")
                    y.append(yt)
                for mt in range(NDT):
                    for nh in range(NH):
                        ssl = slice(nh * 512, (nh + 1) * 512)
                        ps = pp_mm.tile([P, 512], fp32, tag="mm", name="wops")
                        for pr in range(NPAIR):
                            mm(
                                ps,
                                wo_t[:, pr, mt, :],
                                ctx_tiles[pr][:, ssl],
                                start=(pr == 0),
                                stop=(pr == NPAIR - 1),
                            )
                        nc.vector.tensor_add(y[mt][:, ssl], ps, xt[mt][:, ssl])

                def layernorm(yl, g_t, be_t, out_tiles=None):
                    # LN over the partition (feature) dim of the 4 feature-
                    # tiles in yl. Stats via ones-matmuls; the two nh-half
                    # chains are interleaved so DVE/ACT latency pipelines,
                    # and the normalize runs as [P,1024] whole-row ops.
                    p1l, p2l, meanl, rstdl = [], [], [], []
                    for nh in range(NH):
                        ssl = slice(nh * 512, (nh + 1) * 512)
                        pool, tg = (pp_mm, "mm") if nh == 0 else (pp_sc, "sc")
                        p1 = pool.tile([1, 512], fp32, tag=tg, name="p1")
                        for dt in range(NDT):
                            mm(
                                p1,
                                ones_col_bf,
                                yl[dt][:, ssl],
                                start=(dt == 0),
                                stop=(dt == NDT - 1),
                            )
                        p2 = pool.tile([1, 512], fp32, tag=tg, name="p2")
                        for dt in range(NDT):
                            sq = ysq_p.tile([P, 512], bf16, tag="ysq", name="sq")
                            nc.vector.tensor_mul(sq, yl[dt][:, ssl], yl[dt][:, ssl])
                            mm(
                                p2,
                                ones_col_bf,
                                sq,
                                start=(dt == 0),
                                stop=(dt == NDT - 1),
                            )
                        p1l.append(p1)
                        p2l.append(p2)
                    for nh in range(NH):
                        mean = rows_p.tile(
                            [1, 512], fp32r, tag=f"mean{nh}", bufs=1, name="mean"
                        )
                        nc.vector.tensor_scalar_mul(mean, p1l[nh], 1.0 / D)
                        meanl.append(mean)
                    varl = []
                    for nh in range(NH):
                        var = rows_p.tile(
                            [1, 512], fp32, tag=f"var{nh}", bufs=1, name="var"
                        )
                        nc.vector.tensor_mul(var, f(meanl[nh]), f(meanl[nh]))
                        nc.vector.scalar_tensor_tensor(
                            var, p2l[nh], 1.0 / D, var, OP.mult, OP.subtract
                        )
                        varl.append(var)
                    for nh in range(NH):
                        # rstd = exp(-0.5 * ln(var + eps))
                        nc.scalar.activation(
                            varl[nh], varl[nh], AF.Ln, bias=eps_col[0:1]
                        )
                    for nh in range(NH):
                        rstd = rows_p.tile(
                            [1, 512], fp32r, tag=f"rstd{nh}", bufs=1, name="rstd"
                        )
                        nc.scalar.activation(
                            rstd, varl[nh], AF.Exp, bias=zero_col[0:1], scale=-0.5
                        )
                        rstdl.append(rstd)
                    bcm_sb = bcs_p.tile([P, 1024], bf16, tag="bcs", name="bcmsb")
                    bcr_sb = bcs_p.tile([P, 1024], bf16, tag="bcs", name="bcrsb")
                    for nh in range(NH):
                        ssl = slice(nh * 512, (nh + 1) * 512)
                        pool, tg = (pp_mm, "mm") if nh == 0 else (pp_sc, "sc")
                        bcm = pool.tile([P, 512], fp32, tag=tg, name="bcm")
                        mmr(bcm, ones_full[0:1, :], meanl[nh])
                        bcr = pool.tile([P, 512], fp32, tag=tg, name="bcr")
                        mmr(bcr, ones_full[0:1, :], rstdl[nh])
                        if copy_on_act:
                            nc.scalar.copy(bcm_sb[:, ssl], bcm)
                            nc.scalar.copy(bcr_sb[:, ssl], bcr)
                        else:
                            nc.vector.tensor_copy(bcm_sb[:, ssl], bcm)
                            nc.vector.tensor_copy(bcr_sb[:, ssl], bcr)
                    for dt in range(NDT):
                        yr = yl[dt][:, 0:S]
                        dst = yr if out_tiles is None else out_tiles[dt][:, 0:S]
                        eng = nc.gpsimd if dt >= 2 and out_tiles is None else nc.vector
                        eng.tensor_sub(yr, yr, bcm_sb)
                        eng.tensor_mul(yr, yr, bcr_sb)
                        eng.tensor_scalar(
                            out=dst,
                            in0=yr,
                            scalar1=g_t[:, dt : dt + 1],
                            scalar2=be_t[:, dt : dt + 1],
                            op0=OP.mult,
                            op1=OP.add,
                        )

                layernorm(y, g1_t, be1_t)  # y is now mhaT

                # ---------------- FFN ---------------------------------------
                z = []
                for mt in range(NDT):
                    zt = xt_p.tile([P, S], bf16, tag="xt", name="zt",
                                   padded_shape=[P, 2 * S])
                    z.append(zt)
                for nh in range(NH):
                    ssl = slice(nh * 512, (nh + 1) * 512)
                    ff2_ps = []
                    for mt in range(NDT):
                        pool = pp_acc if mt < 2 else pp_sc
                        tag = "acc" if mt < 2 else "sc"
                        ff2_ps.append(
                            pool.tile([P, 512], fp32, tag=tag, name="ff2ps")
                        )
                    pending = None  # ft awaiting FFN2 emission
                    for ft in range(NFT):
                        ps = pp_mm.tile([P, 512], fp32, tag="mm", name="f1ps")
                        for dt in range(NDT):
                            mm(
                                ps,
                                w1_t[:, dt, ft, :],
                                y[dt][:, ssl],
                                start=(dt == 0),
                                stop=(dt == NDT - 1),
                            )
                        f1 = ff1_p.tile([P, 512], bf16, tag="ff1", name="f1")
                        if relu_on_act:
                            nc.scalar.activation(
                                f1, ps, AF.Relu, bias=b1_t[:, ft : ft + 1]
                            )
                        else:
                            nc.vector.tensor_scalar(
                                out=f1,
                                in0=ps,
                                scalar1=b1_t[:, ft : ft + 1],
                                scalar2=0.0,
                                op0=OP.add,
                                op1=OP.max,
                            )
                        if pending is not None:
                            pft, pf1 = pending
                            for mt in range(NDT):
                                mm(
                                    ff2_ps[mt],
                                    w2_t[:, pft, mt, :],
                                    pf1,
                                    start=(pft == 0),
                                    stop=False,
                                )
                        pending = (ft, f1)
                    pft, pf1 = pending
                    for mt in range(NDT):
                        mm(
                            ff2_ps[mt],
                            w2_t[:, pft, mt, :],
                            pf1,
                            start=False,
                            stop=True,
                        )
                        nc.vector.scalar_tensor_tensor(
                            z[mt][:, ssl],
                            ff2_ps[mt],
                            b2_t[:, mt : mt + 1],
                            y[mt][:, ssl],
                            OP.add,
                            OP.add,
                        )

                if l < L - 1:
                    layernorm(z, g2_t, be2_t)  # z is now next layer's xT
                    xt = z
                else:
                    ots = [
                        out_p.tile([P, S], fp32, tag="out", name="ot")
                        for _ in range(NDT)
                    ]
                    layernorm(z, g2_t, be2_t, out_tiles=ots)
                    for dt in range(NDT):
                        nc.sync.dma_start(out=out_d[dt], in_=ots[dt])

    return nc


def _prep_weights(Wq, Wk, Wv, Wo, ln1_g, ln1_b, W1, b1, W2, b2, ln2_g, ln2_b):
    f = np.float32
    bf = _bf16()

    def qk_r(W):  # [L,H,D,DK] -> [L, 128, NDT, NPAIR, 128]
        return np.ascontiguousarray(
            W.reshape(L, NPAIR, 2, NDT, P, DK)
            .transpose(0, 4, 3, 1, 2, 5)
            .reshape(L, P, NDT, NPAIR, P)
            .astype(bf)
        )

    wv_r = np.ascontiguousarray(
        Wv.transpose(0, 2, 1, 3)  # [L, D, H, DK]
        .reshape(L, NDT, P, H * DK)
        .transpose(0, 2, 1, 3)
        .reshape(L, P, NDT, H * DK)
        .astype(bf)
    )
    # Wo packed for K=128 pair-steps: [l, (head01, dk)=128, pr, mt, f]
    wo_r = np.ascontiguousarray(
        Wo.reshape(L, NPAIR, P, NDT, P).transpose(0, 2, 1, 3, 4).astype(bf)
    )
    w1_r = np.ascontiguousarray(
        W1.reshape(L, NDT, P, NFT, P).transpose(0, 2, 1, 3, 4).astype(bf)
    )
    w2_r = np.ascontiguousarray(
        W2.reshape(L, NFT, P, NDT, P).transpose(0, 2, 1, 3, 4).astype(bf)
    )

    def ln_r(v, n):  # [L, n*128] -> [L, 128, n]
        return np.ascontiguousarray(
            v.reshape(L, n, P).transpose(0, 2, 1).astype(f)
        )

    return {
        "wq": qk_r(Wq),
        "wk": qk_r(Wk),
        "wv": wv_r,
        "wo": wo_r,
        "w1": w1_r,
        "w2": w2_r,
        "g1": ln_r(ln1_g, NDT),
        "be1": ln_r(ln1_b, NDT),
        "g2": ln_r(ln2_g, NDT),
        "be2": ln_r(ln2_b, NDT),
        "b1": ln_r(b1, NFT),
        "b2": ln_r(b2, NDT),
    }


def get_nc():
    if "nc" not in _CACHE:
        nc = _build_nc()
        if not nc.is_finalized():
            nc.finalize()
        _CACHE["nc"] = nc
    return _CACHE["nc"]


def make_in_maps(**inputs):
    inputs = {k: np.asarray(v, dtype=np.float32) for k, v in inputs.items()}
    x = inputs.pop("x")
    wmap = _prep_weights(**inputs)
    in_maps = []
    wmap["ones"] = np.ones((P, P), dtype=np.float32)
    bf = _bf16()
    for b in range(B):
        xt = np.ascontiguousarray(x[b].T.reshape(NDT, P, S).astype(bf))
        in_maps.append({"x": xt, **wmap})
    return in_maps


def kernel(**inputs) -> np.ndarray:
    from concourse.bass_utils import run_bass_kernel_spmd

    nc = get_nc()
    in_maps = make_in_maps(**inputs)
    res = run_bass_kernel_spmd(nc, in_maps, core_ids=list(range(B)))
    out = np.empty((B, S, D), dtype=np.float32)
    for b in range(B):
        out[b] = res.results[b]["out"].reshape(D, S).T
    return out


if __name__ == "__main__":
    rng = np.random.default_rng(0)
    ins = {
        "x": rng.standard_normal((B, S, D), dtype=np.float32),
        "Wq": rng.standard_normal((L, H, D, DK), dtype=np.float32) * 0.02,
        "Wk": rng.standard_normal((L, H, D, DK), dtype=np.float32) * 0.02,
        "Wv": rng.standard_normal((L, H, D, DK), dtype=np.float32) * 0.02,
        "Wo": rng.standard_normal((L, D, D), dtype=np.float32) * 0.02,
        "ln1_g": np.ones((L, D), np.float32),
        "ln1_b": np.zeros((L, D), np.float32),
        "W1": rng.standard_normal((L, D, DFF), dtype=np.float32) * 0.02,
        "b1": np.zeros((L, DFF), np.float32),
        "W2": rng.standard_normal((L, DFF, D), dtype=np.float32) * 0.02,
        "b2": np.zeros((L, D), np.float32),
        "ln2_g": np.ones((L, D), np.float32),
        "ln2_b": np.zeros((L, D), np.float32),
    }
    out = kernel(**ins)
    print(out.shape, out.dtype, np.abs(out).mean())


# revision 24
# speedup vs baseline: 1.7367x; 1.0695x over previous
"""Trainium2 Bass kernel for a 6-layer post-LN transformer encoder.

Sharding: data-parallel over batch — B=8, one batch element per NeuronCore,
no collectives.  Each core runs the full 6-layer encoder on its [S, D] slice.

Device-side layout: activations are kept feature-major ([D, S], "xT") in SBUF
so that every matmul can use the natural input-major weights as the stationary
(lhsT) operand and PE contracts over the partition dim:

  out[m, n] = sum_k lhsT[k, m] * rhs[k, n]

Attention is computed transposed (scoresT[t, s]) so softmax needs no
transposes: denominators fall out of a ones-column in the ctx matmul, and the
per-column 1/denom broadcast is a k=1 matmul on PE.

v2 changes vs baseline:
  * bf16 activations + weights (fp32 PSUM accumulate).  2x DVE throughput on
    SBUF elementwise ops, half the DMA traffic.  Small stats rows stay fp32r.
  * Score matmuls for the two heads of a pair are emitted adjacently: K=64
    row-group tiling (auto tile_position (0,0)/(64,0)) runs them concurrently.
  * Both heads' scores land in one [P,1024] PSUM tile -> a single Exp
    activation per (pair, nh, t) halves ACT instruction overhead.
  * ctx for head B uses an M=128 stationary [ones|0(63)|V_B] so its rows land
    at partitions 64..127 (denominator at row 0): the per-pair ctx tile is
    [128, S] and Wo contracts K=128 (half the Wo matmuls).
  * Software-pipelined emission: scores(t+1) ahead of ctx(t), FFN1(ft) ahead
    of FFN2(ft-1), QK of pair p+1 between the two nh halves of pair p.
  * relu / PSUM->SBUF broadcast copies split between ScalarE and DVE.
"""

import numpy as np

L, H, D, DK, DFF = 6, 8, 512, 64, 2048
B, S = 8, 1024
EPS = 1e-5
P = 128
NDT = D // P        # 4  d-tiles
NST = S // P        # 8  s/t-tiles
NFT = DFF // P      # 16 dff-tiles
NPAIR = H // 2      # 4  head pairs
NH = S // 512       # 2  n-halves (512-wide fp32 matmul free dim)
FCH = 2             # W1 streamed in chunks of 2 dff-tiles
VBLK = 196          # per-pair V block: [V_A(64)|1|.|1|zeros|V_B(64) @130]
SCALE = 1.0 / np.sqrt(np.float32(DK))

_CACHE = {}


def _bf16():
    from concourse import mybir

    return mybir.dt.np(mybir.dt.bfloat16)


def _build_nc():
    import concourse.bass as bass
    import concourse.bacc as bacc
    import concourse.tile as tile
    from concourse import mybir

    fp32 = mybir.dt.float32
    fp32r = mybir.dt.float32r
    bf16 = mybir.dt.bfloat16
    AF = mybir.ActivationFunctionType
    OP = mybir.AluOpType

    class _Bacc(bacc.Bacc):
        # Exp (softmax) and Ln (layernorm rstd) live in different default
        # activation-table sets, causing ~50 table-load thrashes (~2.7us
        # each). Restrict both to natural_log_exp_and_others (which holds
        # both) so one load serves the whole kernel. Positional set ids are
        # preserved; only the function->set resolution changes.
        def insert_act_table_loads(self):
            from concourse.hw_specs import get_activation_tables
            import bass_rust as _bass_rust

            has_act = any(
                isinstance(i, mybir.InstActivation)
                for b in self.main_func.blocks
                for i in b.instructions
            )
            if not has_act:
                return
            AF2 = mybir.ActivationFunctionType
            tables = []
            for name, fns in get_activation_tables(self.m.arch).items():
                if name != "natural_log_exp_and_others":
                    fns = fns - {AF2.Exp, AF2.Ln}
                tables.append((name, fns))
            _bass_rust.insert_act_table_loads(self, tables)

    nc = _Bacc()

    from concourse.hw_specs import get_activation_tables

    _nl_set = get_activation_tables(nc.m.arch).get(
        "natural_log_exp_and_others", set()
    )
    relu_on_act = AF.Relu in _nl_set and AF.Copy in _nl_set
    copy_on_act = AF.Copy in _nl_set

    def mm(out, lhsT, rhs, **kw):
        return nc.tensor.matmul(out, lhsT, rhs, **kw)

    def mmr(out, lhsT, rhs, **kw):
        # fp32r matmul for the small stats/broadcast rows
        return nc.tensor.matmul(
            out, lhsT.bitcast(fp32r), rhs.bitcast(fp32r), **kw
        )

    def f(ap):
        # view a float32r tile as plain fp32 for DVE/ACT reads
        return ap.bitcast(fp32)

    x_d = nc.declare_dram_parameter("x", [NDT, P, S], bf16, isOutput=False)
    wq_d = nc.declare_dram_parameter(
        "wq", [L, P, NDT, NPAIR, P], bf16, isOutput=False
    )
    wk_d = nc.declare_dram_parameter(
        "wk", [L, P, NDT, NPAIR, P], bf16, isOutput=False
    )
    wv_d = nc.declare_dram_parameter("wv", [L, P, NDT, H * DK], bf16, isOutput=False)
    wo_d = nc.declare_dram_parameter(
        "wo", [L, P, NPAIR, NDT, P], bf16, isOutput=False
    )
    w1_d = nc.declare_dram_parameter(
        "w1", [L, P, NDT, NFT, P], bf16, isOutput=False
    )
    w2_d = nc.declare_dram_parameter(
        "w2", [L, P, NFT, NDT, P], bf16, isOutput=False
    )
    g1_d = nc.declare_dram_parameter("g1", [L, P, NDT], fp32, isOutput=False)
    be1_d = nc.declare_dram_parameter("be1", [L, P, NDT], fp32, isOutput=False)
    g2_d = nc.declare_dram_parameter("g2", [L, P, NDT], fp32, isOutput=False)
    be2_d = nc.declare_dram_parameter("be2", [L, P, NDT], fp32, isOutput=False)
    b1_d = nc.declare_dram_parameter("b1", [L, P, NFT], fp32, isOutput=False)
    b2_d = nc.declare_dram_parameter("b2", [L, P, NDT], fp32, isOutput=False)
    ones_d = nc.declare_dram_parameter("ones", [P, P], fp32r, isOutput=False)
    out_d = nc.declare_dram_parameter("out", [NDT, P, S], fp32, isOutput=True)

    with tile.TileContext(nc) as tc:
        from contextlib import ExitStack

        with ExitStack() as ctx:
            ec = ctx.enter_context
            ec(
                nc.allow_low_precision(
                    reason="bf16 matmul operands; fp32 PSUM accumulation"
                )
            )
            # --- SBUF pools ---
            const_p = ec(tc.tile_pool(name="const", bufs=1))
            wts_p = ec(tc.tile_pool(name="wts", bufs=2))
            xt_p = ec(tc.tile_pool(name="xt", bufs=4))
            qk_p = ec(tc.tile_pool(name="qk", bufs=2))
            v_p = ec(tc.tile_pool(name="v", bufs=8))
            exp_p = ec(tc.tile_pool(name="exp", bufs=4))
            ctx_p = ec(tc.tile_pool(name="ctxp", bufs=4))
            mha_p = ec(tc.tile_pool(name="mha", bufs=4))
            ff1_p = ec(tc.tile_pool(name="ff1", bufs=4))
            ysq_p = ec(tc.tile_pool(name="ysq", bufs=2))
            bcs_p = ec(tc.tile_pool(name="bcs", bufs=2))
            rows_p = ec(tc.tile_pool(name="rows", bufs=1))
            out_p = ec(tc.tile_pool(name="outp", bufs=4))
            # --- PSUM pools: 4 + 2 + 2 = 8 banks ---
            # sc:  [P,1024] scores (A|B) tiles; FFN borrows [P,512] slots
            # acc: attention ctx accumulators (A, B)
            # mm:  short-lived matmul outputs (QKV/V/Wo/FFN1/LN stats+bc)
            pp_sc = ec(tc.tile_pool(name="pp_sc", bufs=2, space="PSUM"))
            pp_acc = ec(tc.tile_pool(name="pp_acc", bufs=2, space="PSUM"))
            pp_mm = ec(tc.tile_pool(name="pp_mm", bufs=2, space="PSUM"))

            # ones come from DRAM so the fp32r data counts as pre-rounded
            ones_full = const_p.tile([P, P], fp32r)
            nc.sync.dma_start(out=ones_full, in_=ones_d[:, :])
            ones_col_bf = const_p.tile([P, 1], bf16)
            nc.vector.memset(ones_col_bf, 1.0)
            # selector rows for the paired 1/denom broadcast:
            #   selA row: [1]*64 + [0]*64   selB row: [0]*64 + [1]*64
            selA_t = const_p.tile([P, P], fp32r)
            nc.vector.memset(f(selA_t)[:, 0:64], 1.0)
            nc.vector.memset(f(selA_t)[:, 64:128], 0.0)
            selB_t = const_p.tile([P, P], fp32r)
            nc.vector.memset(f(selB_t)[:, 0:64], 0.0)
            nc.vector.memset(f(selB_t)[:, 64:128], 1.0)
            zero_col = const_p.tile([P, 1], fp32)
            nc.vector.memset(zero_col, 0.0)
            eps_col = const_p.tile([P, 1], fp32)
            nc.vector.memset(eps_col, float(EPS))

            # layer-0 input
            xt = []
            for dt in range(NDT):
                t = xt_p.tile([P, S], bf16, tag="xt")
                nc.sync.dma_start(out=t, in_=x_d[dt])
                xt.append(t)

            def make_qk(pr, w_t, tag):
                dst = qk_p.tile([P, S], bf16, tag=tag, name="qkdst")
                for nh in range(NH):
                    ps = pp_mm.tile([P, 512], fp32, tag="mm", name="qkps")
                    for dt in range(NDT):
                        mm(
                            ps,
                            w_t[:, dt, pr, :],
                            xt[dt][:, nh * 512 : (nh + 1) * 512],
                            start=(dt == 0),
                            stop=(dt == NDT - 1),
                        )
                    nc.vector.tensor_copy(dst[:, nh * 512 : (nh + 1) * 512], ps)
                return dst

            for l in range(L):
                # ---------------- weight loads (bufs=2 pools: next layer's
                # loads overlap this layer's compute) ------------------------
                wq_t = wts_p.tile([P, NDT, NPAIR, P], bf16, tag="wq")
                nc.sync.dma_start(out=wq_t, in_=wq_d[l])
                wk_t = wts_p.tile([P, NDT, NPAIR, P], bf16, tag="wk")
                nc.sync.dma_start(out=wk_t, in_=wk_d[l])
                wv_t = wts_p.tile([P, NDT, H * DK], bf16, tag="wv")
                nc.sync.dma_start(out=wv_t, in_=wv_d[l])
                wo_t = wts_p.tile([P, NPAIR, NDT, P], bf16, tag="wo")
                nc.sync.dma_start(out=wo_t, in_=wo_d[l])
                g1_t = wts_p.tile([P, NDT], fp32, tag="g1")
                nc.sync.dma_start(out=g1_t, in_=g1_d[l])
                be1_t = wts_p.tile([P, NDT], fp32, tag="be1")
                nc.sync.dma_start(out=be1_t, in_=be1_d[l])
                g2_t = wts_p.tile([P, NDT], fp32, tag="g2")
                nc.sync.dma_start(out=g2_t, in_=g2_d[l])
                be2_t = wts_p.tile([P, NDT], fp32, tag="be2")
                nc.sync.dma_start(out=be2_t, in_=be2_d[l])
                b1_t = wts_p.tile([P, NFT], fp32, tag="b1")
                nc.sync.dma_start(out=b1_t, in_=b1_d[l])
                b2_t = wts_p.tile([P, NDT], fp32, tag="b2")
                nc.sync.dma_start(out=b2_t, in_=b2_d[l])
                w1_t = wts_p.tile([P, NDT, NFT, P], bf16, tag="w1")
                nc.sync.dma_start(out=w1_t, in_=w1_d[l])
                w2_t = wts_p.tile([P, NFT, NDT, P], bf16, tag="w2")
                nc.sync.dma_start(out=w2_t, in_=w2_d[l])

                # ---------------- Q/K for pair 0 ----------------------------
                qt = [None] * NPAIR
                kt = [None] * NPAIR
                qt[0] = make_qk(0, wq_t, "qt")
                kt[0] = make_qk(0, wk_t, "kt")

                # ---------------- V = x @ Wv, packed per head pair ----------
                # vt[:, pr, 0:64]    = V of head 2*pr       (ctx rows 0..63)
                # vt[:, pr, 64]      = 1                    (denom A, row 64)
                # vt[:, pr, 66]      = 1                    (denom B, row 0)
                # vt[:, pr, 67:130]  = 0                    (junk rows 1..63)
                # vt[:, pr, 130:194] = V of head 2*pr+1     (ctx rows 64..127)
                v_tiles = []
                for st in range(NST):
                    vt = v_p.tile([P, NPAIR, VBLK], bf16, tag="v")
                    ps = pp_mm.tile([P, 512], fp32, tag="mm", name="vps")
                    for dt in range(NDT):
                        mm(
                            ps,
                            xt[dt][:, st * P : (st + 1) * P],
                            wv_t[:, dt, :],
                            start=(dt == 0),
                            stop=(dt == NDT - 1),
                        )
                    psh = ps.rearrange("p (h k) -> p h k", h=H)
                    nc.vector.tensor_copy(vt[:, :, 0:DK], psh[:, 0::2, :])
                    nc.vector.tensor_copy(vt[:, :, 130 : 130 + DK], psh[:, 1::2, :])
                    nc.gpsimd.memset(vt[:, :, 67:130], 0.0)
                    nc.gpsimd.memset(vt[:, :, 64:65], 1.0)
                    nc.gpsimd.memset(vt[:, :, 66:67], 1.0)
                    v_tiles.append(vt)

                # ---------------- attention ---------------------------------
                ctx_tiles = []
                for pr in range(NPAIR):
                    ch = ctx_p.tile([P, S], bf16, tag="ctx", name="ch")
                    ctx_tiles.append(ch)

                def scores(pr, nh, t):
                    ssl = slice(nh * 512, (nh + 1) * 512)
                    tsl = slice(t * P, (t + 1) * P)
                    sc = pp_sc.tile([P, 1024], fp32, tag="sc", name="sc")
                    # two K=64 matmuls on distinct row groups -> concurrent
                    mm(sc[:, 0:512], kt[pr][0:64, tsl], qt[pr][0:64, ssl])
                    mm(sc[:, 512:1024], kt[pr][64:128, tsl], qt[pr][64:128, ssl])
                    return sc

                def attend(pr, nh):
                    ssl = slice(nh * 512, (nh + 1) * 512)
                    psA = pp_acc.tile([P, 512], fp32, tag="acc", name="psA")
                    psB = pp_acc.tile([P, 512], fp32, tag="acc", name="psB")
                    sc_cur = scores(pr, nh, 0)
                    for t in range(NST):
                        sc_next = scores(pr, nh, t + 1) if t + 1 < NST else None
                        e = exp_p.tile([P, 1024], bf16, tag="exp", name="e")
                        nc.scalar.activation(
                            e, sc_cur, AF.Exp, bias=zero_col, scale=float(SCALE)
                        )
                        vt = v_tiles[t]
                        mm(
                            psA[0:65, :],
                            vt[:, pr, 0 : DK + 1],
                            e[:, 0:512],
                            start=(t == 0),
                            stop=(t == NST - 1),
                        )
                        mm(
                            psB,
                            vt[:, pr, 66:194],
                            e[:, 512:1024],
                            start=(t == 0),
                            stop=(t == NST - 1),
                        )
                        sc_cur = sc_next
                    # normalize: ctx rows / denominator (A: row 64, B: row 0).
                    # ScalarE copies the raw ctx rows out first so the acc
                    # PSUM banks release early (next pair's ctx can start);
                    # the 1/denom scale then runs in-place in SBUF with the
                    # broadcast read straight from PSUM.
                    rA = rows_p.tile([65, 512], fp32r, tag="rA", bufs=2, name="rA")
                    nc.vector.reciprocal(rA[64:65], psA[64:65])
                    rB = rows_p.tile([1, 512], fp32r, tag="rB", bufs=2, name="rB")
                    nc.vector.reciprocal(rB, psB[0:1])
                    ch = ctx_tiles[pr]
                    nc.vector.tensor_copy(ch[0:64, ssl], psA[0:64])
                    nc.vector.tensor_copy(ch[64:128, ssl], psB[64:128])
                    # bc rows 0..63 = 1/dA, rows 64..127 = 1/dB via selector
                    # rows (two accumulating M=128 matmuls, no col tiling)
                    bc = pp_mm.tile([P, 512], fp32, tag="mm", name="bc")
                    mmr(bc, selA_t[64:65, :], rA[64:65], start=True, stop=False)
                    mmr(bc, selB_t[0:1, :], rB, start=False, stop=True)
                    nc.vector.tensor_mul(ch[0:64, ssl], ch[0:64, ssl], bc[0:64])
                    nc.vector.tensor_mul(ch[64:128, ssl], ch[64:128, ssl], bc[64:128])

                for pr in range(NPAIR):
                    attend(pr, 0)
                    if pr + 1 < NPAIR:
                        # PE work to cover the normalize tail / acc release
                        qt[pr + 1] = make_qk(pr + 1, wq_t, "qt")
                        kt[pr + 1] = make_qk(pr + 1, wk_t, "kt")
                    attend(pr, 1)

                # ---------------- Wo + residual -> y (pre-LN1) --------------
                y = []
                for mt in range(NDT):
                    yt = mha_p.tile([P, S], bf16, tag="mha", name="yt")
                    y.append(yt)
                for mt in range(NDT):
                    for nh in range(NH):
                        ssl = slice(nh * 512, (nh + 1) * 512)
                        ps = pp_mm.tile([P, 512], fp32, tag="mm", name="wops")
                        for pr in range(NPAIR):
                            mm(
                                ps,
                                wo_t[:, pr, mt, :],
                                ctx_tiles[pr][:, ssl],
                                start=(pr == 0),
                                stop=(pr == NPAIR - 1),
                            )
                        nc.vector.tensor_add(y[mt][:, ssl], ps, xt[mt][:, ssl])

                def layernorm(yl, g_t, be_t, out_tiles=None):
                    # LN over the partition (feature) dim of the 4 feature-
                    # tiles in yl. Stats via ones-matmuls; the two nh-half
                    # chains are interleaved so DVE/ACT latency pipelines,
                    # and the normalize runs as [P,1024] whole-row ops.
                    p1l, p2l, meanl, rstdl = [], [], [], []
                    for nh in range(NH):
                        ssl = slice(nh * 512, (nh + 1) * 512)
                        pool, tg = (pp_mm, "mm") if nh == 0 else (pp_sc, "sc")
                        p1 = pool.tile([1, 512], fp32, tag=tg, name="p1")
                        for dt in range(NDT):
                            mm(
                                p1,
                                ones_col_bf,
                                yl[dt][:, ssl],
                                start=(dt == 0),
                                stop=(dt == NDT - 1),
                            )
                        p2 = pool.tile([1, 512], fp32, tag=tg, name="p2")
                        for dt in range(NDT):
                            sq = ysq_p.tile([P, 512], bf16, tag="ysq", name="sq")
                            nc.vector.tensor_mul(sq, yl[dt][:, ssl], yl[dt][:, ssl])
                            mm(
                                p2,
                                ones_col_bf,
                                sq,
                                start=(dt == 0),
                                stop=(dt == NDT - 1),
                            )
                        p1l.append(p1)
                        p2l.append(p2)
                    for nh in range(NH):
                        mean = rows_p.tile(
                            [1, 512], fp32r, tag=f"mean{nh}", bufs=1, name="mean"
                        )
                        nc.vector.tensor_scalar_mul(mean, p1l[nh], 1.0 / D)
                        meanl.append(mean)
                    varl = []
                    for nh in range(NH):
                        var = rows_p.tile(
                            [1, 512], fp32, tag=f"var{nh}", bufs=1, name="var"
                        )
                        nc.vector.tensor_mul(var, f(meanl[nh]), f(meanl[nh]))
                        nc.vector.scalar_tensor_tensor(
                            var, p2l[nh], 1.0 / D, var, OP.mult, OP.subtract
                        )
                        varl.append(var)
                    for nh in range(NH):
                        # rstd = exp(-0.5 * ln(var + eps))
                        nc.scalar.activation(
                            varl[nh], varl[nh], AF.Ln, bias=eps_col[0:1]
                        )
                    for nh in range(NH):
                        rstd = rows_p.tile(
                            [1, 512], fp32r, tag=f"rstd{nh}", bufs=1, name="rstd"
                        )
                        nc.scalar.activation(
                            rstd, varl[nh], AF.Exp, bias=zero_col[0:1], scale=-0.5
                        )
                        rstdl.append(rstd)
                    bcm_sb = bcs_p.tile([P, 1024], bf16, tag="bcs", name="bcmsb")
                    bcr_sb = bcs_p.tile([P, 1024], bf16, tag="bcs", name="bcrsb")
                    for nh in range(NH):
                        ssl = slice(nh * 512, (nh + 1) * 512)
                        pool, tg = (pp_mm, "mm") if nh == 0 else (pp_sc, "sc")
                        bcm = pool.tile([P, 512], fp32, tag=tg, name="bcm")
                        mmr(bcm, ones_full[0:1, :], meanl[nh])
                        bcr = pool.tile([P, 512], fp32, tag=tg, name="bcr")
                        mmr(bcr, ones_full[0:1, :], rstdl[nh])
                        if copy_on_act:
                            nc.scalar.copy(bcm_sb[:, ssl], bcm)
                            nc.scalar.copy(bcr_sb[:, ssl], bcr)
                        else:
                            nc.vector.tensor_copy(bcm_sb[:, ssl], bcm)
                            nc.vector.tensor_copy(bcr_sb[:, ssl], bcr)
                    for dt in range(NDT):
                        for nh in range(NH):
                            ssl = slice(nh * 512, (nh + 1) * 512)
                            yr = yl[dt][:, ssl]
                            dst = (
                                yr
                                if out_tiles is None
                                else out_tiles[dt][:, ssl]
                            )
                            eng = (
                                nc.gpsimd
                                if dt >= 2 and out_tiles is None
                                else nc.vector
                            )
                            eng.tensor_sub(yr, yr, bcm_sb[:, ssl])
                            eng.tensor_mul(yr, yr, bcr_sb[:, ssl])
                            eng.tensor_scalar(
                                out=dst,
                                in0=yr,
                                scalar1=g_t[:, dt : dt + 1],
                                scalar2=be_t[:, dt : dt + 1],
                                op0=OP.mult,
                                op1=OP.add,
                            )

                layernorm(y, g1_t, be1_t)  # y is now mhaT

                # ---------------- FFN ---------------------------------------
                z = []
                for mt in range(NDT):
                    zt = xt_p.tile([P, S], bf16, tag="xt", name="zt",
                                   padded_shape=[P, 2 * S])
                    z.append(zt)
                for nh in range(NH):
                    ssl = slice(nh * 512, (nh + 1) * 512)
                    ff2_ps = []
                    for mt in range(NDT):
                        pool = pp_acc if mt < 2 else pp_sc
                        tag = "acc" if mt < 2 else "sc"
                        ff2_ps.append(
                            pool.tile([P, 512], fp32, tag=tag, name="ff2ps")
                        )
                    pending = None  # ft awaiting FFN2 emission
                    for ft in range(NFT):
                        ps = pp_mm.tile([P, 512], fp32, tag="mm", name="f1ps")
                        for dt in range(NDT):
                            mm(
                                ps,
                                w1_t[:, dt, ft, :],
                                y[dt][:, ssl],
                                start=(dt == 0),
                                stop=(dt == NDT - 1),
                            )
                        f1 = ff1_p.tile([P, 512], bf16, tag="ff1", name="f1")
                        if relu_on_act:
                            nc.scalar.activation(
                                f1, ps, AF.Relu, bias=b1_t[:, ft : ft + 1]
                            )
                        else:
                            nc.vector.tensor_scalar(
                                out=f1,
                                in0=ps,
                                scalar1=b1_t[:, ft : ft + 1],
                                scalar2=0.0,
                                op0=OP.add,
                                op1=OP.max,
                            )
                        if pending is not None:
                            pft, pf1 = pending
                            for mt in range(NDT):
                                mm(
                                    ff2_ps[mt],
                                    w2_t[:, pft, mt, :],
                                    pf1,
                                    start=(pft == 0),
                                    stop=False,
                                )
                        pending = (ft, f1)
                    pft, pf1 = pending
                    for mt in range(NDT):
                        mm(
                            ff2_ps[mt],
                            w2_t[:, pft, mt, :],
                            pf1,
                            start=False,
                            stop=True,
                        )
                        nc.vector.scalar_tensor_tensor(
                            z[mt][:, ssl],
                            ff2_ps[mt],
                            b2_t[:, mt : mt + 1],
                            y[mt][:, ssl],
                            OP.add,
                            OP.add,
                        )

                if l < L - 1:
                    layernorm(z, g2_t, be2_t)  # z is now next layer's xT
                    xt = z
                else:
                    ots = [
                        out_p.tile([P, S], fp32, tag="out", name="ot")
                        for _ in range(NDT)
                    ]
                    layernorm(z, g2_t, be2_t, out_tiles=ots)
                    for dt in range(NDT):
                        nc.sync.dma_start(out=out_d[dt], in_=ots[dt])

    return nc


def _prep_weights(Wq, Wk, Wv, Wo, ln1_g, ln1_b, W1, b1, W2, b2, ln2_g, ln2_b):
    f = np.float32
    bf = _bf16()

    def qk_r(W):  # [L,H,D,DK] -> [L, 128, NDT, NPAIR, 128]
        return np.ascontiguousarray(
            W.reshape(L, NPAIR, 2, NDT, P, DK)
            .transpose(0, 4, 3, 1, 2, 5)
            .reshape(L, P, NDT, NPAIR, P)
            .astype(bf)
        )

    wv_r = np.ascontiguousarray(
        Wv.transpose(0, 2, 1, 3)  # [L, D, H, DK]
        .reshape(L, NDT, P, H * DK)
        .transpose(0, 2, 1, 3)
        .reshape(L, P, NDT, H * DK)
        .astype(bf)
    )
    # Wo packed for K=128 pair-steps: [l, (head01, dk)=128, pr, mt, f]
    wo_r = np.ascontiguousarray(
        Wo.reshape(L, NPAIR, P, NDT, P).transpose(0, 2, 1, 3, 4).astype(bf)
    )
    w1_r = np.ascontiguousarray(
        W1.reshape(L, NDT, P, NFT, P).transpose(0, 2, 1, 3, 4).astype(bf)
    )
    w2_r = np.ascontiguousarray(
        W2.reshape(L, NFT, P, NDT, P).transpose(0, 2, 1, 3, 4).astype(bf)
    )

    def ln_r(v, n):  # [L, n*128] -> [L, 128, n]
        return np.ascontiguousarray(
            v.reshape(L, n, P).transpose(0, 2, 1).astype(f)
        )

    return {
        "wq": qk_r(Wq),
        "wk": qk_r(Wk),
        "wv": wv_r,
        "wo": wo_r,
        "w1": w1_r,
        "w2": w2_r,
        "g1": ln_r(ln1_g, NDT),
        "be1": ln_r(ln1_b, NDT),
        "g2": ln_r(ln2_g, NDT),
        "be2": ln_r(ln2_b, NDT),
        "b1": ln_r(b1, NFT),
        "b2": ln_r(b2, NDT),
    }


def get_nc():
    if "nc" not in _CACHE:
        nc = _build_nc()
        if not nc.is_finalized():
            nc.finalize()
        _CACHE["nc"] = nc
    return _CACHE["nc"]


def make_in_maps(**inputs):
    inputs = {k: np.asarray(v, dtype=np.float32) for k, v in inputs.items()}
    x = inputs.pop("x")
    wmap = _prep_weights(**inputs)
    in_maps = []
    wmap["ones"] = np.ones((P, P), dtype=np.float32)
    bf = _bf16()
    for b in range(B):
        xt = np.ascontiguousarray(x[b].T.reshape(NDT, P, S).astype(bf))
        in_maps.append({"x": xt, **wmap})
    return in_maps


def kernel(**inputs) -> np.ndarray:
    from concourse.bass_utils import run_bass_kernel_spmd

    nc = get_nc()
    in_maps = make_in_maps(**inputs)
    res = run_bass_kernel_spmd(nc, in_maps, core_ids=list(range(B)))
    out = np.empty((B, S, D), dtype=np.float32)
    for b in range(B):
        out[b] = res.results[b]["out"].reshape(D, S).T
    return out


if __name__ == "__main__":
    rng = np.random.default_rng(0)
    ins = {
        "x": rng.standard_normal((B, S, D), dtype=np.float32),
        "Wq": rng.standard_normal((L, H, D, DK), dtype=np.float32) * 0.02,
        "Wk": rng.standard_normal((L, H, D, DK), dtype=np.float32) * 0.02,
        "Wv": rng.standard_normal((L, H, D, DK), dtype=np.float32) * 0.02,
        "Wo": rng.standard_normal((L, D, D), dtype=np.float32) * 0.02,
        "ln1_g": np.ones((L, D), np.float32),
        "ln1_b": np.zeros((L, D), np.float32),
        "W1": rng.standard_normal((L, D, DFF), dtype=np.float32) * 0.02,
        "b1": np.zeros((L, DFF), np.float32),
        "W2": rng.standard_normal((L, DFF, D), dtype=np.float32) * 0.02,
        "b2": np.zeros((L, D), np.float32),
        "ln2_g": np.ones((L, D), np.float32),
        "ln2_b": np.zeros((L, D), np.float32),
    }
    out = kernel(**ins)
    print(out.shape, out.dtype, np.abs(out).mean())
